# revision 19
# baseline (speedup 1.0000x reference)
"""nn_AttentionHead kernel for 8 Trainium2 NeuronCores.

Sharding: data-parallel over batch (16 batches -> 2 per core). phi/V/LN params
replicated; the [n,n] score matrix stays core-local.

Device pipeline per core (2 batches, n=1024, d=128), all heavy math on device:
  norms/xhat, Y = U @ xhat^T (scaled), scores^T = Y^T... (j,i)-layout,
  exp, rowsums via ones-matmul, attn@V in feature-major, transpose back,
  residual + LayerNorm.  Score/AV matmuls run in fp32r (TF32-class, 4x faster
  than fp32 on the PE); everything else fp32.

U(phi) is a fixed reparameterization of the phi parameter vector (8128 Givens
angles -> one orthogonal 128x128 matrix); it is prepared host-side in float64
and shipped as an input, like any other weight-layout preprocessing.
"""

import numpy as np

D = 128
SEQ = 1024
BATCH = 16
N_CORES = 8
B_PER_CORE = BATCH // N_CORES          # 2
TILES = B_PER_CORE * SEQ // 128        # 16 token tiles per core
TPB = SEQ // 128                       # 8 token tiles per batch
EPS_LN = 1e-5

# cst blob column layout
C_UT = 0          # U^T              [128, 128]
C_EYE = 128       # identity         [128, 128]
C_VWT = 256       # Vw^T             [128, 128]
C_GAM = 384       # gamma replicated [128, 128]
C_BET = 512       # beta replicated  [128, 128]
C_VBR = 640       # Vb replicated    [128, 128]
C_ONE = 768       # ones column      [128, 1]
C_W = 769


def _build_U(phi: np.ndarray) -> np.ndarray:
    d = D
    U = np.eye(d, dtype=np.float64)
    p = phi.astype(np.float64)
    k = 0
    for i in range(1, d):
        for j in range(i, 0, -1):
            a, b = j - 1, j
            c, s = np.cos(p[k]), np.sin(p[k])
            ra = U[a].copy()
            rb = U[b].copy()
            U[a] = c * ra + s * rb
            U[b] = -s * ra + c * rb
            k += 1
    return U


_CACHE = {}


def _build_program(debug=False):
    import concourse.bass as bass
    import concourse.tile as tile
    import concourse.mybir as mybir
    from concourse import bacc

    AF = mybir.ActivationFunctionType
    ALU = mybir.AluOpType
    f32 = mybir.dt.float32
    f32r = mybir.dt.float32r

    nc = bacc.Bacc(None, target_bir_lowering=False, num_devices=N_CORES)
    dbg = {}
    xin = nc.dram_tensor("xin", [B_PER_CORE * SEQ, D], f32, kind="ExternalInput").ap()
    cst = nc.dram_tensor("cst", [128, C_W], f32, kind="ExternalInput").ap()
    yout = nc.dram_tensor("yout", [B_PER_CORE * SEQ, D], f32, kind="ExternalOutput").ap()

    with tile.TileContext(nc) as tc:
        with (
            tc.tile_pool(name="big", bufs=1) as big,
            tc.tile_pool(name="work", bufs=2) as work,
            tc.tile_pool(name="ps512", bufs=2, space="PSUM") as ps512,
        ):
            # ---- loads ----
            cstt = big.tile([128, C_W], f32)
            nc.gpsimd.dma_start(cstt[:], cst[:])
            xrow = big.tile([128, TILES * 128], f32)  # token-major x, all tiles
            for bb in range(B_PER_CORE):
                nc.sync.dma_start(
                    xrow[:, bb * TPB * 128:(bb + 1) * TPB * 128]
                    .rearrange("p (t c) -> p t c", c=128),
                    xin[bb * SEQ:(bb + 1) * SEQ, :].rearrange("(t p) c -> p t c", p=128),
                )
            UT = cstt[:, C_UT:C_UT + 128]
            EYE = cstt[:, C_EYE:C_EYE + 128]
            VWT = cstt[:, C_VWT:C_VWT + 128]
            GAM = cstt[:, C_GAM:C_GAM + 128]
            BET = cstt[:, C_BET:C_BET + 128]
            VBR = cstt[:, C_VBR:C_VBR + 128]
            ONE = cstt[:, C_ONE:C_ONE + 1]

            ones_r = big.tile([128, 1], f32r)
            nc.vector.tensor_copy(ones_r[:], ONE)
            onesm = big.tile([128, 128], f32)
            nc.gpsimd.memset(onesm[:], 1.0)
            onesm_r = big.tile([128, 128], f32r)
            nc.vector.tensor_copy(onesm_r[:], onesm[:])
            vwt_r = big.tile([128, 128], f32r)
            nc.vector.tensor_copy(vwt_r[:], VWT)

            # ---- per-token stats: normsq via ACT Square accum_out ----
            NS = big.tile([128, TILES], f32)       # sum of squares per token
            for t in range(TILES):
                sc = work.tile([128, 128], f32, tag="sqs", name=f"sqs{t}")
                nc.scalar.activation(sc[:], xrow[:, t * 128:(t + 1) * 128],
                                     AF.Square)
                nc.vector.reduce_sum(NS[:, t:t + 1], sc[:],
                                     axis=mybir.AxisListType.X)
            # sqrt-free stats: everything via exp(k*ln(NS)) so all ACT funcs
            # stay in one table set (ln/exp/square/copy)
            nc.vector.tensor_scalar_max(NS[:], NS[:], 1e-24)
            LNS = big.tile([128, TILES], f32)
            nc.scalar.activation(LNS[:], NS[:], AF.Ln)
            rinv = big.tile([128, TILES], f32)     # NS**-0.5
            nc.scalar.activation(rinv[:], LNS[:], AF.Exp, scale=-0.5)
            rY = big.tile([128, TILES], f32)       # NS**-0.25 * 128**-0.25
            nc.scalar.activation(rY[:], LNS[:], AF.Exp, scale=-0.25)
            nc.vector.tensor_scalar_mul(rY[:], rY[:], float(D ** -0.25))
            norms = big.tile([128, TILES], f32)    # NS**0.5
            nc.vector.tensor_tensor(out=norms[:], in0=NS[:], in1=rinv[:], op=ALU.mult)

            # ---- scaled rows + transposes ----
            xhat_t = big.tile([128, TILES * 128], f32r)   # xhat^T  (d-major)
            xs_t = big.tile([128, TILES * 128], f32)      # scaled  (d-major)
            for half, (dst, scl, dt_) in enumerate(
                [(xhat_t, rinv, f32r), (xs_t, rY, f32)]
            ):
                for q in range(TILES // 4):       # quads of 4 tiles
                    pt = ps512.tile([128, 512], f32, tag="q5", bufs=1, name=f"trq{half}_{q}")
                    for k in range(4):
                        t = q * 4 + k
                        row = work.tile([128, 128], f32, tag="rowsc",
                                        name=f"rs{half}_{t}")
                        nc.vector.tensor_scalar_mul(
                            row[:], xrow[:, t * 128:(t + 1) * 128], scl[:, t:t + 1])
                        nc.tensor.transpose(pt[:, k * 128:(k + 1) * 128], row[:], EYE)
                    nc.vector.tensor_copy(dst[:, q * 512:(q + 1) * 512], pt[:])

            # ---- Y^T = U @ xs^T  (fp32), evac to fp32r ----
            y_t = big.tile([128, TILES * 128], f32r)
            for q in range(TILES // 4):
                yp = ps512.tile([128, 512], f32, tag="q5", bufs=1, name=f"yq{q}")
                nc.tensor.matmul(yp[:], UT, xs_t[:, q * 512:(q + 1) * 512],
                                 start=True, stop=True)
                nc.scalar.copy(y_t[:, q * 512:(q + 1) * 512], yp[:])

            # ---- V0 = x @ Vw^T  (token-major, fused norms scale on evac) ----
            v_r = big.tile([128, TILES * 128], f32r)
            for t in range(TILES):
                vp = ps512.tile([128, 128], f32, tag="sm", bufs=1, name=f"vp{t}")
                nc.tensor.matmul(vp[:], xhat_t[:, t * 128:(t + 1) * 128], vwt_r[:],
                                 start=True, stop=True)
                nc.vector.tensor_scalar_mul(v_r[:, t * 128:(t + 1) * 128], vp[:],
                                            norms[:, t:t + 1])

            # ---- XV = x + Vb (residual base) ----
            xv = big.tile([128, TILES * 128], f32)
            for t in range(TILES):
                nc.gpsimd.tensor_tensor(
                    out=xv[:, t * 128:(t + 1) * 128],
                    in0=xrow[:, t * 128:(t + 1) * 128], in1=VBR, op=ALU.add)

            OUT = big.tile([128, TILES * 128], f32)
            hblob = big.tile([128, TILES * 128], f32)
            MU = big.tile([128, TILES], f32)
            SQ = big.tile([128, TILES], f32)
            zcols = big.tile([128, TILES], f32)

            # ================= per-batch attention =================
            for b in range(B_PER_CORE):
                jbase = b * TPB
                et = [None] * TPB
                # scores^T tiles [j_tile, i] + exp
                for jt in range(TPB):
                    ett = work.tile([128, SEQ], f32r, tag="et", bufs=10,
                                    name=f"et{b}_{jt}")
                    et[jt] = ett
                    stp = ps512.tile([128, 1024], f32, tag="qst", bufs=1, name=f"st{b}_{jt}")
                    for ic in range(2):
                        nc.tensor.matmul(
                            stp[:, ic * 512:(ic + 1) * 512],
                            y_t[:, (jbase + jt) * 128:(jbase + jt + 1) * 128],
                            xhat_t[:, b * SEQ + ic * 512: b * SEQ + (ic + 1) * 512],
                            start=True, stop=True)
                    asb = work.tile([128, 1024], f32, tag="asb", bufs=3,
                                    name=f"asb{b}_{jt}")
                    nc.scalar.activation(asb[:], stp[:], AF.Square)
                    nc.scalar.activation(ett[:], asb[:], AF.Exp)
                # attn @ V (feature-major) + rowsums, accumulated over j tiles
                htp = [None, None]
                for ic in range(2):
                    htp[ic] = ps512.tile([128, 512], f32, tag=f"ht{ic}", bufs=1,
                                         name=f"ht{b}_{ic}")
                zzp = ps512.tile([128, 1024], f32, tag="zzp", bufs=1, name=f"zzp{b}")
                zp = [zzp[:, 0:512], zzp[:, 512:1024]]
                for jt in range(TPB):
                    for ic in range(2):
                        nc.tensor.matmul(
                            htp[ic][:], v_r[:, (jbase + jt) * 128:(jbase + jt + 1) * 128],
                            et[jt][:, ic * 512:(ic + 1) * 512],
                            start=(jt == 0), stop=(jt == TPB - 1))
                        nc.tensor.matmul(
                            zp[ic], onesm_r[:],
                            et[jt][:, ic * 512:(ic + 1) * 512],
                            start=(jt == 0), stop=(jt == TPB - 1))
                # evac Ht + Z
                ht_sb = work.tile([128, SEQ], f32, tag="htsb", bufs=2, name=f"htsb{b}")
                zrow = work.tile([1, SEQ], f32, tag="zrow", bufs=2, name=f"zrow{b}")
                for ic in range(2):
                    nc.scalar.copy(ht_sb[:, ic * 512:(ic + 1) * 512], htp[ic][:])
                nc.scalar.copy(zrow[:], zzp[:1, :])
                # redistribute Z row -> columns [128, TPB] via K=1 matmuls
                ztp = ps512.tile([128, TPB], f32, tag="sm", bufs=1, name=f"ztp{b}")
                for it in range(TPB):
                    nc.tensor.matmul(ztp[:, it:it + 1],
                                     zrow[:1, it * 128:(it + 1) * 128],
                                     ONE[:1, :1], start=(it == 0), stop=(it == TPB - 1))
                nc.scalar.copy(zcols[:, b * TPB:(b + 1) * TPB], ztp[:])
                # transpose Ht back to token-major, build h = Ht^T/Z + XV
                rz = work.tile([128, TPB], f32, tag="rz", bufs=2, name=f"rz{b}")
                nc.vector.reciprocal(rz[:], zcols[:, b * TPB:(b + 1) * TPB])
                for hq in range(2):
                    hp = ps512.tile([128, 512], f32, tag="q5", bufs=1, name=f"hq{b}_{hq}")
                    for k in range(4):
                        it = hq * 4 + k
                        nc.tensor.transpose(hp[:, k * 128:(k + 1) * 128],
                                            ht_sb[:, it * 128:(it + 1) * 128], EYE)
                    for k in range(4):
                        it = hq * 4 + k
                        t = b * TPB + it
                        hsc = work.tile([128, 128], f32, tag="hsc", bufs=2,
                                        name=f"hsc{b}_{hq}_{k}")
                        nc.vector.tensor_scalar_mul(hsc[:], hp[:, k * 128:(k + 1) * 128],
                                                    rz[:, it:it + 1])
                        nc.vector.tensor_tensor(
                            out=hblob[:, t * 128:(t + 1) * 128], in0=hsc[:],
                            in1=xv[:, t * 128:(t + 1) * 128], op=ALU.add)

                # ---- LayerNorm for this batch's 8 tiles ----
                bs, be = b * TPB, (b + 1) * TPB
                for t in range(bs, be):
                    hs = hblob[:, t * 128:(t + 1) * 128]
                    nc.vector.reduce_sum(MU[:, t:t + 1], hs, axis=mybir.AxisListType.X)
                    sc2 = work.tile([128, 128], f32, tag="sqs2", bufs=2, name=f"sq2_{t}")
                    nc.vector.tensor_tensor(out=sc2[:], in0=hs, in1=hs, op=ALU.mult)
                    nc.vector.reduce_sum(SQ[:, t:t + 1], sc2[:], axis=mybir.AxisListType.X)
                MEAN = MU[:, bs:be]
                SQb = SQ[:, bs:be]
                msq = work.tile([128, TPB], f32, tag="msq", bufs=2, name=f"msq{b}")
                VAR = work.tile([128, TPB], f32, tag="var", bufs=2, name=f"var{b}")
                nc.vector.tensor_scalar_mul(MEAN, MU[:, bs:be], 1.0 / D)
                nc.vector.tensor_tensor(out=msq[:], in0=MEAN, in1=MEAN, op=ALU.mult)
                nc.vector.tensor_scalar(VAR[:], SQb, 1.0 / D, None, ALU.mult)
                nc.vector.tensor_tensor(out=VAR[:], in0=VAR[:], in1=msq[:], op=ALU.subtract)
                nc.vector.tensor_scalar_add(VAR[:], VAR[:], EPS_LN)
                LV = work.tile([128, TPB], f32, tag="std", bufs=2, name=f"std{b}")
                nc.scalar.activation(LV[:], VAR[:], AF.Ln)
                RSTD = work.tile([128, TPB], f32, tag="rstd", bufs=2, name=f"rstd{b}")
                nc.scalar.activation(RSTD[:], LV[:], AF.Exp, scale=-0.5)
                for t in range(bs, be):
                    nc.vector.tensor_scalar(
                        OUT[:, t * 128:(t + 1) * 128], hblob[:, t * 128:(t + 1) * 128],
                        MEAN[:, t - bs:t - bs + 1], RSTD[:, t - bs:t - bs + 1],
                        ALU.subtract, ALU.mult)
                for t in range(bs, be):
                    sl = OUT[:, t * 128:(t + 1) * 128]
                    nc.gpsimd.tensor_tensor(out=sl, in0=sl, in1=GAM, op=ALU.mult)
                    nc.gpsimd.tensor_tensor(out=sl, in0=sl, in1=BET, op=ALU.add)
                nc.sync.dma_start(
                    yout[b * SEQ:(b + 1) * SEQ, :].rearrange("(t p) c -> p t c", p=128),
                    OUT[:, bs * 128:be * 128].rearrange("p (t c) -> p t c", c=128),
                )


            if debug:
                for nm, tl in [("d_xhat", xhat_t), ("d_y", y_t), ("d_v", v_r),
                               ("d_h", hblob), ("d_et", et[0]), ("d_ht", ht_sb),
                               ("d_zr", zrow)]:
                    dd = nc.dram_tensor(nm, list(tl.shape), f32,
                                        kind="ExternalOutput").ap()
                    if tl[:].dtype == f32r:
                        tmp = big.tile(list(tl.shape), f32, name=f"cv_{nm}")
                        nc.vector.tensor_copy(tmp[:], tl[:].bitcast(f32))
                        nc.gpsimd.dma_start(dd[:], tmp[:])
                    else:
                        nc.gpsimd.dma_start(dd[:], tl[:])
                dz = nc.dram_tensor("d_z", [128, TILES], f32, kind="ExternalOutput").ap()
                nc.gpsimd.dma_start(dz[:], zcols[:])
    nc.compile()
    return nc


def _get_nc():
    if "nc" not in _CACHE:
        _CACHE["nc"] = _build_program()
    return _CACHE["nc"]


def kernel(x, phi, Vw, Vb, gamma, beta):
    from concourse.bass_utils import run_bass_kernel_spmd

    x = np.asarray(x, dtype=np.float32)
    U = _build_U(np.asarray(phi)).astype(np.float32)

    cst = np.zeros((128, C_W), dtype=np.float32)
    cst[:, C_UT:C_UT + 128] = U.T
    cst[:, C_EYE:C_EYE + 128] = np.eye(128, dtype=np.float32)
    cst[:, C_VWT:C_VWT + 128] = np.asarray(Vw, np.float32).T
    cst[:, C_GAM:C_GAM + 128] = np.broadcast_to(np.asarray(gamma, np.float32), (128, 128))
    cst[:, C_BET:C_BET + 128] = np.broadcast_to(np.asarray(beta, np.float32), (128, 128))
    cst[:, C_VBR:C_VBR + 128] = np.broadcast_to(np.asarray(Vb, np.float32), (128, 128))
    cst[:, C_ONE] = 1.0

    nc = _get_nc()
    in_maps = []
    for c in range(N_CORES):
        xs = x[c * B_PER_CORE:(c + 1) * B_PER_CORE].reshape(B_PER_CORE * SEQ, D)
        in_maps.append({"xin": np.ascontiguousarray(xs), "cst": cst})
    res = run_bass_kernel_spmd(nc, in_maps, core_ids=list(range(N_CORES)))
    out = np.empty((BATCH, SEQ, D), dtype=np.float32)
    for c in range(N_CORES):
        out[c * B_PER_CORE:(c + 1) * B_PER_CORE] = (
            res.results[c]["yout"].reshape(B_PER_CORE, SEQ, D))
    return out


# revision 20
# speedup vs baseline: 1.0121x; 1.0121x over previous
"""nn_AttentionHead kernel for 8 Trainium2 NeuronCores.

Sharding: data-parallel over batch (16 batches -> 2 per core). phi/V/LN params
replicated; the [n,n] score matrix stays core-local.

Device pipeline per core (2 batches, n=1024, d=128), all heavy math on device:
  norms/xhat, Y = U @ xhat^T (scaled), scores^T = Y^T... (j,i)-layout,
  exp, rowsums via ones-matmul, attn@V in feature-major, transpose back,
  residual + LayerNorm.  Score/AV matmuls run in fp32r (TF32-class, 4x faster
  than fp32 on the PE); everything else fp32.

U(phi) is a fixed reparameterization of the phi parameter vector (8128 Givens
angles -> one orthogonal 128x128 matrix); it is prepared host-side in float64
and shipped as an input, like any other weight-layout preprocessing.
"""

import numpy as np

D = 128
SEQ = 1024
BATCH = 16
N_CORES = 8
B_PER_CORE = BATCH // N_CORES          # 2
TILES = B_PER_CORE * SEQ // 128        # 16 token tiles per core
TPB = SEQ // 128                       # 8 token tiles per batch
EPS_LN = 1e-5

# cst blob column layout
C_UT = 0          # U^T              [128, 128]
C_EYE = 128       # identity         [128, 128]
C_VWT = 256       # Vw^T             [128, 128]
C_GAM = 384       # gamma replicated [128, 128]
C_BET = 512       # beta replicated  [128, 128]
C_VBR = 640       # Vb replicated    [128, 128]
C_ONE = 768       # ones column      [128, 1]
C_W = 769


def _build_U(phi: np.ndarray) -> np.ndarray:
    d = D
    U = np.eye(d, dtype=np.float64)
    p = phi.astype(np.float64)
    k = 0
    for i in range(1, d):
        for j in range(i, 0, -1):
            a, b = j - 1, j
            c, s = np.cos(p[k]), np.sin(p[k])
            ra = U[a].copy()
            rb = U[b].copy()
            U[a] = c * ra + s * rb
            U[b] = -s * ra + c * rb
            k += 1
    return U


_CACHE = {}


def _build_program(debug=False):
    import concourse.bass as bass
    import concourse.tile as tile
    import concourse.mybir as mybir
    from concourse import bacc

    AF = mybir.ActivationFunctionType
    ALU = mybir.AluOpType
    f32 = mybir.dt.float32
    f32r = mybir.dt.float32r

    nc = bacc.Bacc(None, target_bir_lowering=False, num_devices=N_CORES)
    dbg = {}
    xin = nc.dram_tensor("xin", [B_PER_CORE * SEQ, D], f32, kind="ExternalInput").ap()
    cst = nc.dram_tensor("cst", [128, C_W], f32, kind="ExternalInput").ap()
    yout = nc.dram_tensor("yout", [B_PER_CORE * SEQ, D], f32, kind="ExternalOutput").ap()

    with tile.TileContext(nc) as tc:
        with (
            tc.tile_pool(name="big", bufs=1) as big,
            tc.tile_pool(name="work", bufs=2) as work,
            tc.tile_pool(name="ps512", bufs=2, space="PSUM") as ps512,
        ):
            # ---- loads ----
            cstt = big.tile([128, C_W], f32)
            nc.gpsimd.dma_start(cstt[:], cst[:])
            xrow = big.tile([128, TILES * 128], f32)  # token-major x, all tiles
            for bb in range(B_PER_CORE):
                nc.sync.dma_start(
                    xrow[:, bb * TPB * 128:(bb + 1) * TPB * 128]
                    .rearrange("p (t c) -> p t c", c=128),
                    xin[bb * SEQ:(bb + 1) * SEQ, :].rearrange("(t p) c -> p t c", p=128),
                )
            UT = cstt[:, C_UT:C_UT + 128]
            EYE = cstt[:, C_EYE:C_EYE + 128]
            VWT = cstt[:, C_VWT:C_VWT + 128]
            GAM = cstt[:, C_GAM:C_GAM + 128]
            BET = cstt[:, C_BET:C_BET + 128]
            VBR = cstt[:, C_VBR:C_VBR + 128]
            ONE = cstt[:, C_ONE:C_ONE + 1]

            ones_r = big.tile([128, 1], f32r)
            nc.vector.tensor_copy(ones_r[:], ONE)
            onesm = big.tile([128, 128], f32)
            nc.gpsimd.memset(onesm[:], 1.0)
            onesm_r = big.tile([128, 128], f32r)
            nc.vector.tensor_copy(onesm_r[:], onesm[:])
            vwt_r = big.tile([128, 128], f32r)
            nc.vector.tensor_copy(vwt_r[:], VWT)

            # ---- per-token stats: normsq via ACT Square accum_out ----
            NS = big.tile([128, TILES], f32)       # sum of squares per token
            for t in range(TILES):
                sc = work.tile([128, 128], f32, tag="sqs", name=f"sqs{t}")
                nc.scalar.activation(sc[:], xrow[:, t * 128:(t + 1) * 128],
                                     AF.Square)
                nc.vector.reduce_sum(NS[:, t:t + 1], sc[:],
                                     axis=mybir.AxisListType.X)
            norms = big.tile([128, TILES], f32)
            nc.scalar.activation(norms[:], NS[:], AF.Sqrt)
            normc = big.tile([128, TILES], f32)    # max(norms, eps)
            nc.vector.tensor_scalar_max(normc[:], norms[:], 1e-12)
            rinv = big.tile([128, TILES], f32)     # 1 / max(norms, eps)
            nc.vector.reciprocal(rinv[:], normc[:])
            rY = big.tile([128, TILES], f32)       # sqrt(rinv) * 128**-0.25
            nc.scalar.activation(rY[:], rinv[:], AF.Sqrt, scale=float(D ** -0.5))

            # ---- scaled rows + transposes ----
            xhat_t = big.tile([128, TILES * 128], f32r)   # xhat^T  (d-major)
            xs_t = big.tile([128, TILES * 128], f32)      # scaled  (d-major)
            for half, (dst, scl, dt_) in enumerate(
                [(xhat_t, rinv, f32r), (xs_t, rY, f32)]
            ):
                for q in range(TILES // 4):       # quads of 4 tiles
                    pt = ps512.tile([128, 512], f32, tag="q5", bufs=1, name=f"trq{half}_{q}")
                    for k in range(4):
                        t = q * 4 + k
                        row = work.tile([128, 128], f32, tag="rowsc",
                                        name=f"rs{half}_{t}")
                        nc.vector.tensor_scalar_mul(
                            row[:], xrow[:, t * 128:(t + 1) * 128], scl[:, t:t + 1])
                        nc.tensor.transpose(pt[:, k * 128:(k + 1) * 128], row[:], EYE)
                    nc.vector.tensor_copy(dst[:, q * 512:(q + 1) * 512], pt[:])

            # ---- Y^T = U @ xs^T  (fp32), evac to fp32r ----
            y_t = big.tile([128, TILES * 128], f32r)
            for q in range(TILES // 4):
                yp = ps512.tile([128, 512], f32, tag="q5", bufs=1, name=f"yq{q}")
                nc.tensor.matmul(yp[:], UT, xs_t[:, q * 512:(q + 1) * 512],
                                 start=True, stop=True)
                nc.scalar.copy(y_t[:, q * 512:(q + 1) * 512], yp[:])

            # ---- V0 = x @ Vw^T  (token-major, fused norms scale on evac) ----
            v_r = big.tile([128, TILES * 128], f32r)
            for t in range(TILES):
                vp = ps512.tile([128, 128], f32, tag="sm", bufs=1, name=f"vp{t}")
                nc.tensor.matmul(vp[:], xhat_t[:, t * 128:(t + 1) * 128], vwt_r[:],
                                 start=True, stop=True)
                nc.vector.tensor_scalar_mul(v_r[:, t * 128:(t + 1) * 128], vp[:],
                                            norms[:, t:t + 1])

            # ---- XV = x + Vb (residual base) ----
            xv = big.tile([128, TILES * 128], f32)
            for t in range(TILES):
                nc.gpsimd.tensor_tensor(
                    out=xv[:, t * 128:(t + 1) * 128],
                    in0=xrow[:, t * 128:(t + 1) * 128], in1=VBR, op=ALU.add)

            OUT = big.tile([128, TILES * 128], f32)
            hblob = big.tile([128, TILES * 128], f32)
            MU = big.tile([128, TILES], f32)
            SQ = big.tile([128, TILES], f32)
            zcols = big.tile([128, TILES], f32)

            # ================= per-batch attention =================
            for b in range(B_PER_CORE):
                jbase = b * TPB
                et = [None] * TPB
                # scores^T tiles [j_tile, i] + exp
                for jt in range(TPB):
                    ett = work.tile([128, SEQ], f32r, tag="et", bufs=10,
                                    name=f"et{b}_{jt}")
                    et[jt] = ett
                    stp = ps512.tile([128, 1024], f32, tag="qst", bufs=1, name=f"st{b}_{jt}")
                    for ic in range(2):
                        nc.tensor.matmul(
                            stp[:, ic * 512:(ic + 1) * 512],
                            y_t[:, (jbase + jt) * 128:(jbase + jt + 1) * 128],
                            xhat_t[:, b * SEQ + ic * 512: b * SEQ + (ic + 1) * 512],
                            start=True, stop=True)
                    asb = work.tile([128, 1024], f32, tag="asb", bufs=3,
                                    name=f"asb{b}_{jt}")
                    nc.scalar.activation(asb[:], stp[:], AF.Square)
                    nc.scalar.activation(ett[:], asb[:], AF.Exp)
                # attn @ V (feature-major) + rowsums, accumulated over j tiles
                htp = [None, None]
                for ic in range(2):
                    htp[ic] = ps512.tile([128, 512], f32, tag=f"ht{ic}", bufs=1,
                                         name=f"ht{b}_{ic}")
                zzp = ps512.tile([128, 1024], f32, tag="zzp", bufs=1, name=f"zzp{b}")
                zp = [zzp[:, 0:512], zzp[:, 512:1024]]
                for jt in range(TPB):
                    for ic in range(2):
                        nc.tensor.matmul(
                            htp[ic][:], v_r[:, (jbase + jt) * 128:(jbase + jt + 1) * 128],
                            et[jt][:, ic * 512:(ic + 1) * 512],
                            start=(jt == 0), stop=(jt == TPB - 1))
                        nc.tensor.matmul(
                            zp[ic], onesm_r[:],
                            et[jt][:, ic * 512:(ic + 1) * 512],
                            start=(jt == 0), stop=(jt == TPB - 1))
                # evac Ht + Z
                ht_sb = work.tile([128, SEQ], f32, tag="htsb", bufs=2, name=f"htsb{b}")
                zrow = work.tile([1, SEQ], f32, tag="zrow", bufs=2, name=f"zrow{b}")
                for ic in range(2):
                    nc.scalar.copy(ht_sb[:, ic * 512:(ic + 1) * 512], htp[ic][:])
                nc.scalar.copy(zrow[:], zzp[:1, :])
                # redistribute Z row -> columns [128, TPB] via K=1 matmuls
                ztp = ps512.tile([128, TPB], f32, tag="sm", bufs=1, name=f"ztp{b}")
                for it in range(TPB):
                    nc.tensor.matmul(ztp[:, it:it + 1],
                                     zrow[:1, it * 128:(it + 1) * 128],
                                     ONE[:1, :1], start=(it == 0), stop=(it == TPB - 1))
                nc.scalar.copy(zcols[:, b * TPB:(b + 1) * TPB], ztp[:])
                # transpose Ht back to token-major, build h = Ht^T/Z + XV
                rz = work.tile([128, TPB], f32, tag="rz", bufs=2, name=f"rz{b}")
                nc.vector.reciprocal(rz[:], zcols[:, b * TPB:(b + 1) * TPB])
                for hq in range(2):
                    hp = ps512.tile([128, 512], f32, tag="q5", bufs=1, name=f"hq{b}_{hq}")
                    for k in range(4):
                        it = hq * 4 + k
                        nc.tensor.transpose(hp[:, k * 128:(k + 1) * 128],
                                            ht_sb[:, it * 128:(it + 1) * 128], EYE)
                    for k in range(4):
                        it = hq * 4 + k
                        t = b * TPB + it
                        hsc = work.tile([128, 128], f32, tag="hsc", bufs=2,
                                        name=f"hsc{b}_{hq}_{k}")
                        nc.vector.tensor_scalar_mul(hsc[:], hp[:, k * 128:(k + 1) * 128],
                                                    rz[:, it:it + 1])
                        nc.vector.tensor_tensor(
                            out=hblob[:, t * 128:(t + 1) * 128], in0=hsc[:],
                            in1=xv[:, t * 128:(t + 1) * 128], op=ALU.add)

                # ---- LayerNorm for this batch's 8 tiles ----
                bs, be = b * TPB, (b + 1) * TPB
                for t in range(bs, be):
                    hs = hblob[:, t * 128:(t + 1) * 128]
                    nc.vector.reduce_sum(MU[:, t:t + 1], hs, axis=mybir.AxisListType.X)
                    sc2 = work.tile([128, 128], f32, tag="sqs2", bufs=2, name=f"sq2_{t}")
                    nc.vector.tensor_tensor(out=sc2[:], in0=hs, in1=hs, op=ALU.mult)
                    nc.vector.reduce_sum(SQ[:, t:t + 1], sc2[:], axis=mybir.AxisListType.X)
                MEAN = MU[:, bs:be]
                SQb = SQ[:, bs:be]
                msq = work.tile([128, TPB], f32, tag="msq", bufs=2, name=f"msq{b}")
                VAR = work.tile([128, TPB], f32, tag="var", bufs=2, name=f"var{b}")
                nc.vector.tensor_scalar_mul(MEAN, MU[:, bs:be], 1.0 / D)
                nc.vector.tensor_tensor(out=msq[:], in0=MEAN, in1=MEAN, op=ALU.mult)
                nc.vector.tensor_scalar(VAR[:], SQb, 1.0 / D, None, ALU.mult)
                nc.vector.tensor_tensor(out=VAR[:], in0=VAR[:], in1=msq[:], op=ALU.subtract)
                nc.vector.tensor_scalar_add(VAR[:], VAR[:], EPS_LN)
                STD = work.tile([128, TPB], f32, tag="std", bufs=2, name=f"std{b}")
                nc.scalar.activation(STD[:], VAR[:], AF.Sqrt)
                RSTD = work.tile([128, TPB], f32, tag="rstd", bufs=2, name=f"rstd{b}")
                nc.vector.reciprocal(RSTD[:], STD[:])
                for t in range(bs, be):
                    nc.vector.tensor_scalar(
                        OUT[:, t * 128:(t + 1) * 128], hblob[:, t * 128:(t + 1) * 128],
                        MEAN[:, t - bs:t - bs + 1], RSTD[:, t - bs:t - bs + 1],
                        ALU.subtract, ALU.mult)
                for t in range(bs, be):
                    sl = OUT[:, t * 128:(t + 1) * 128]
                    nc.gpsimd.tensor_tensor(out=sl, in0=sl, in1=GAM, op=ALU.mult)
                    nc.gpsimd.tensor_tensor(out=sl, in0=sl, in1=BET, op=ALU.add)
                nc.sync.dma_start(
                    yout[b * SEQ:(b + 1) * SEQ, :].rearrange("(t p) c -> p t c", p=128),
                    OUT[:, bs * 128:be * 128].rearrange("p (t c) -> p t c", c=128),
                )


            if debug:
                for nm, tl in [("d_xhat", xhat_t), ("d_y", y_t), ("d_v", v_r),
                               ("d_h", hblob), ("d_et", et[0]), ("d_ht", ht_sb),
                               ("d_zr", zrow)]:
                    dd = nc.dram_tensor(nm, list(tl.shape), f32,
                                        kind="ExternalOutput").ap()
                    if tl[:].dtype == f32r:
                        tmp = big.tile(list(tl.shape), f32, name=f"cv_{nm}")
                        nc.vector.tensor_copy(tmp[:], tl[:].bitcast(f32))
                        nc.gpsimd.dma_start(dd[:], tmp[:])
                    else:
                        nc.gpsimd.dma_start(dd[:], tl[:])
                dz = nc.dram_tensor("d_z", [128, TILES], f32, kind="ExternalOutput").ap()
                nc.gpsimd.dma_start(dz[:], zcols[:])
    nc.compile()
    return nc


def _get_nc():
    if "nc" not in _CACHE:
        _CACHE["nc"] = _build_program()
    return _CACHE["nc"]


def kernel(x, phi, Vw, Vb, gamma, beta):
    from concourse.bass_utils import run_bass_kernel_spmd

    x = np.asarray(x, dtype=np.float32)
    U = _build_U(np.asarray(phi)).astype(np.float32)

    cst = np.zeros((128, C_W), dtype=np.float32)
    cst[:, C_UT:C_UT + 128] = U.T
    cst[:, C_EYE:C_EYE + 128] = np.eye(128, dtype=np.float32)
    cst[:, C_VWT:C_VWT + 128] = np.asarray(Vw, np.float32).T
    cst[:, C_GAM:C_GAM + 128] = np.broadcast_to(np.asarray(gamma, np.float32), (128, 128))
    cst[:, C_BET:C_BET + 128] = np.broadcast_to(np.asarray(beta, np.float32), (128, 128))
    cst[:, C_VBR:C_VBR + 128] = np.broadcast_to(np.asarray(Vb, np.float32), (128, 128))
    cst[:, C_ONE] = 1.0

    nc = _get_nc()
    in_maps = []
    for c in range(N_CORES):
        xs = x[c * B_PER_CORE:(c + 1) * B_PER_CORE].reshape(B_PER_CORE * SEQ, D)
        in_maps.append({"xin": np.ascontiguousarray(xs), "cst": cst})
    res = run_bass_kernel_spmd(nc, in_maps, core_ids=list(range(N_CORES)))
    out = np.empty((BATCH, SEQ, D), dtype=np.float32)
    for c in range(N_CORES):
        out[c * B_PER_CORE:(c + 1) * B_PER_CORE] = (
            res.results[c]["yout"].reshape(B_PER_CORE, SEQ, D))
    return out


# revision 21
# speedup vs baseline: 1.0290x; 1.0166x over previous
"""nn_AttentionHead kernel for 8 Trainium2 NeuronCores.

Sharding: data-parallel over batch (16 batches -> 2 per core). phi/V/LN params
replicated; the [n,n] score matrix stays core-local.

Device pipeline per core (2 batches, n=1024, d=128), all heavy math on device:
  norms/xhat, Y = U @ xhat^T (scaled), scores^T = Y^T... (j,i)-layout,
  exp, rowsums via ones-matmul, attn@V in feature-major, transpose back,
  residual + LayerNorm.  Score/AV matmuls run in fp32r (TF32-class, 4x faster
  than fp32 on the PE); everything else fp32.

U(phi) is a fixed reparameterization of the phi parameter vector (8128 Givens
angles -> one orthogonal 128x128 matrix); it is prepared host-side in float64
and shipped as an input, like any other weight-layout preprocessing.
"""

import numpy as np

D = 128
SEQ = 1024
BATCH = 16
N_CORES = 8
B_PER_CORE = BATCH // N_CORES          # 2
TILES = B_PER_CORE * SEQ // 128        # 16 token tiles per core
TPB = SEQ // 128                       # 8 token tiles per batch
EPS_LN = 1e-5

# cst blob column layout
C_UT = 0          # U^T              [128, 128]
C_EYE = 128       # identity         [128, 128]
C_VWT = 256       # Vw^T             [128, 128]
C_GAM = 384       # gamma replicated [128, 128]
C_BET = 512       # beta replicated  [128, 128]
C_VBR = 640       # Vb replicated    [128, 128]
C_ONE = 768       # ones column      [128, 1]
C_W = 769


def _build_U(phi: np.ndarray) -> np.ndarray:
    d = D
    U = np.eye(d, dtype=np.float64)
    p = phi.astype(np.float64)
    k = 0
    for i in range(1, d):
        for j in range(i, 0, -1):
            a, b = j - 1, j
            c, s = np.cos(p[k]), np.sin(p[k])
            ra = U[a].copy()
            rb = U[b].copy()
            U[a] = c * ra + s * rb
            U[b] = -s * ra + c * rb
            k += 1
    return U


_CACHE = {}


def _build_program(debug=False):
    import concourse.bass as bass
    import concourse.tile as tile
    import concourse.mybir as mybir
    from concourse import bacc

    from concourse.dve_ops import AFFINE_THEN_ADD
    AF = mybir.ActivationFunctionType
    ALU = mybir.AluOpType
    f32 = mybir.dt.float32
    f32r = mybir.dt.float32r

    nc = bacc.Bacc(None, target_bir_lowering=False, num_devices=N_CORES)
    dbg = {}
    xin = nc.dram_tensor("xin", [B_PER_CORE * SEQ, D], f32, kind="ExternalInput").ap()
    cst = nc.dram_tensor("cst", [128, C_W], f32, kind="ExternalInput").ap()
    yout = nc.dram_tensor("yout", [B_PER_CORE * SEQ, D], f32, kind="ExternalOutput").ap()

    with tile.TileContext(nc) as tc:
        with (
            tc.tile_pool(name="big", bufs=1) as big,
            tc.tile_pool(name="work", bufs=2) as work,
            tc.tile_pool(name="ps512", bufs=2, space="PSUM") as ps512,
        ):
            # ---- loads ----
            cstt = big.tile([128, C_W], f32)
            nc.gpsimd.dma_start(cstt[:], cst[:])
            xrow = big.tile([128, TILES * 128], f32)  # token-major x, all tiles
            for bb in range(B_PER_CORE):
                nc.sync.dma_start(
                    xrow[:, bb * TPB * 128:(bb + 1) * TPB * 128]
                    .rearrange("p (t c) -> p t c", c=128),
                    xin[bb * SEQ:(bb + 1) * SEQ, :].rearrange("(t p) c -> p t c", p=128),
                )
            UT = cstt[:, C_UT:C_UT + 128]
            EYE = cstt[:, C_EYE:C_EYE + 128]
            VWT = cstt[:, C_VWT:C_VWT + 128]
            GAM = cstt[:, C_GAM:C_GAM + 128]
            BET = cstt[:, C_BET:C_BET + 128]
            VBR = cstt[:, C_VBR:C_VBR + 128]
            ONE = cstt[:, C_ONE:C_ONE + 1]

            ones_r = big.tile([128, 1], f32r)
            nc.vector.tensor_copy(ones_r[:], ONE)
            onesm = big.tile([128, 128], f32)
            nc.gpsimd.memset(onesm[:], 1.0)
            onesm_r = big.tile([128, 128], f32r)
            nc.vector.tensor_copy(onesm_r[:], onesm[:])
            vwt_r = big.tile([128, 128], f32r)
            nc.vector.tensor_copy(vwt_r[:], VWT)

            # ---- per-token stats: normsq via ACT Square accum_out ----
            NS = big.tile([128, TILES], f32)       # sum of squares per token
            for t in range(TILES):
                sc = work.tile([128, 128], f32, tag="sqs", name=f"sqs{t}")
                nc.scalar.activation(sc[:], xrow[:, t * 128:(t + 1) * 128],
                                     AF.Square)
                nc.vector.reduce_sum(NS[:, t:t + 1], sc[:],
                                     axis=mybir.AxisListType.X)
            norms = big.tile([128, TILES], f32)
            nc.scalar.activation(norms[:], NS[:], AF.Sqrt)
            normc = big.tile([128, TILES], f32)    # max(norms, eps)
            nc.vector.tensor_scalar_max(normc[:], norms[:], 1e-12)
            rinv = big.tile([128, TILES], f32)     # 1 / max(norms, eps)
            nc.vector.reciprocal(rinv[:], normc[:])
            rY = big.tile([128, TILES], f32)       # sqrt(rinv) * 128**-0.25
            nc.scalar.activation(rY[:], rinv[:], AF.Sqrt, scale=float(D ** -0.5))

            # ---- scaled rows + transposes ----
            xhat_t = big.tile([128, TILES * 128], f32r)   # xhat^T  (d-major)
            xs_t = big.tile([128, TILES * 128], f32)      # scaled  (d-major)
            for half, (dst, scl, dt_) in enumerate(
                [(xhat_t, rinv, f32r), (xs_t, rY, f32)]
            ):
                for q in range(TILES // 4):       # quads of 4 tiles
                    pt = ps512.tile([128, 512], f32, tag="q5", bufs=1, name=f"trq{half}_{q}")
                    for k in range(4):
                        t = q * 4 + k
                        row = work.tile([128, 128], f32, tag="rowsc",
                                        name=f"rs{half}_{t}")
                        nc.vector.tensor_scalar_mul(
                            row[:], xrow[:, t * 128:(t + 1) * 128], scl[:, t:t + 1])
                        nc.tensor.transpose(pt[:, k * 128:(k + 1) * 128], row[:], EYE)
                    nc.vector.tensor_copy(dst[:, q * 512:(q + 1) * 512], pt[:])

            # ---- Y^T = U @ xs^T  (fp32), evac to fp32r ----
            y_t = big.tile([128, TILES * 128], f32r)
            for q in range(TILES // 4):
                yp = ps512.tile([128, 512], f32, tag="q5", bufs=1, name=f"yq{q}")
                nc.tensor.matmul(yp[:], UT, xs_t[:, q * 512:(q + 1) * 512],
                                 start=True, stop=True)
                nc.scalar.copy(y_t[:, q * 512:(q + 1) * 512], yp[:])

            # ---- V0 = x @ Vw^T  (token-major, fused norms scale on evac) ----
            v_r = big.tile([128, TILES * 128], f32r)
            for t in range(TILES):
                vp = ps512.tile([128, 128], f32, tag="sm", bufs=1, name=f"vp{t}")
                nc.tensor.matmul(vp[:], xhat_t[:, t * 128:(t + 1) * 128], vwt_r[:],
                                 start=True, stop=True)
                nc.vector.tensor_scalar_mul(v_r[:, t * 128:(t + 1) * 128], vp[:],
                                            norms[:, t:t + 1])

            # ---- XV = x + Vb (residual base) ----
            xv = big.tile([128, TILES * 128], f32)
            for t in range(TILES):
                nc.gpsimd.tensor_tensor(
                    out=xv[:, t * 128:(t + 1) * 128],
                    in0=xrow[:, t * 128:(t + 1) * 128], in1=VBR, op=ALU.add)

            OUT = big.tile([128, TILES * 128], f32)
            hblob = big.tile([128, TILES * 128], f32)
            MU = big.tile([128, TILES], f32)
            SQ = big.tile([128, TILES], f32)
            zcols = big.tile([128, TILES], f32)

            # ================= per-batch attention =================
            for b in range(B_PER_CORE):
                jbase = b * TPB
                et = [None] * TPB
                # scores^T tiles [j_tile, i] + exp
                for jt in range(TPB):
                    ett = work.tile([128, SEQ], f32r, tag="et", bufs=12,
                                    name=f"et{b}_{jt}")
                    et[jt] = ett
                    stp = ps512.tile([128, 1024], f32, tag="qst", bufs=1, name=f"st{b}_{jt}")
                    for ic in range(2):
                        nc.tensor.matmul(
                            stp[:, ic * 512:(ic + 1) * 512],
                            y_t[:, (jbase + jt) * 128:(jbase + jt + 1) * 128],
                            xhat_t[:, b * SEQ + ic * 512: b * SEQ + (ic + 1) * 512],
                            start=True, stop=True)
                    asb = work.tile([128, 1024], f32, tag="asb", bufs=4,
                                    name=f"asb{b}_{jt}")
                    nc.scalar.activation(asb[:], stp[:], AF.Square)
                    nc.scalar.activation(ett[:], asb[:], AF.Exp)
                # attn @ V (feature-major) + rowsums, accumulated over j tiles
                htp = [None, None]
                for ic in range(2):
                    htp[ic] = ps512.tile([128, 512], f32, tag=f"ht{ic}", bufs=1,
                                         name=f"ht{b}_{ic}")
                zzp = ps512.tile([128, 1024], f32, tag="zzp", bufs=1, name=f"zzp{b}")
                zp = [zzp[:, 0:512], zzp[:, 512:1024]]
                for jt in range(TPB):
                    for ic in range(2):
                        nc.tensor.matmul(
                            htp[ic][:], v_r[:, (jbase + jt) * 128:(jbase + jt + 1) * 128],
                            et[jt][:, ic * 512:(ic + 1) * 512],
                            start=(jt == 0), stop=(jt == TPB - 1))
                        nc.tensor.matmul(
                            zp[ic], onesm_r[:],
                            et[jt][:, ic * 512:(ic + 1) * 512],
                            start=(jt == 0), stop=(jt == TPB - 1))
                # evac Ht + Z
                ht_sb = work.tile([128, SEQ], f32, tag="htsb", bufs=2, name=f"htsb{b}")
                zrow = work.tile([1, SEQ], f32, tag="zrow", bufs=2, name=f"zrow{b}")
                for ic in range(2):
                    nc.scalar.copy(ht_sb[:, ic * 512:(ic + 1) * 512], htp[ic][:])
                nc.scalar.copy(zrow[:], zzp[:1, :])
                # redistribute Z row -> columns [128, TPB] via K=1 matmuls
                ztp = ps512.tile([128, TPB], f32, tag="sm", bufs=1, name=f"ztp{b}")
                for it in range(TPB):
                    nc.tensor.matmul(ztp[:, it:it + 1],
                                     zrow[:1, it * 128:(it + 1) * 128],
                                     ONE[:1, :1], start=(it == 0), stop=(it == TPB - 1))
                nc.scalar.copy(zcols[:, b * TPB:(b + 1) * TPB], ztp[:])
                # transpose Ht back to token-major, build h = Ht^T/Z + XV
                rz = work.tile([128, TPB], f32, tag="rz", bufs=2, name=f"rz{b}")
                nc.vector.reciprocal(rz[:], zcols[:, b * TPB:(b + 1) * TPB])
                for hq in range(2):
                    hp = ps512.tile([128, 512], f32, tag="q5", bufs=1, name=f"hq{b}_{hq}")
                    for k in range(4):
                        it = hq * 4 + k
                        nc.tensor.transpose(hp[:, k * 128:(k + 1) * 128],
                                            ht_sb[:, it * 128:(it + 1) * 128], EYE)
                    for k in range(4):
                        it = hq * 4 + k
                        t = b * TPB + it
                        nc.vector._custom_dve(
                            AFFINE_THEN_ADD,
                            out=hblob[:, t * 128:(t + 1) * 128],
                            in0=hp[:, k * 128:(k + 1) * 128],
                            in1=xv[:, t * 128:(t + 1) * 128],
                            s0=rz[:, it:it + 1], s1=0.0)

                # ---- LayerNorm for this batch's 8 tiles ----
                bs, be = b * TPB, (b + 1) * TPB
                for t in range(bs, be):
                    hs = hblob[:, t * 128:(t + 1) * 128]
                    nc.vector.reduce_sum(MU[:, t:t + 1], hs, axis=mybir.AxisListType.X)
                    sc2 = work.tile([128, 128], f32, tag="sqs2", bufs=2, name=f"sq2_{t}")
                    nc.vector.tensor_tensor(out=sc2[:], in0=hs, in1=hs, op=ALU.mult)
                    nc.vector.reduce_sum(SQ[:, t:t + 1], sc2[:], axis=mybir.AxisListType.X)
                MEAN = MU[:, bs:be]
                SQb = SQ[:, bs:be]
                msq = work.tile([128, TPB], f32, tag="msq", bufs=2, name=f"msq{b}")
                VAR = work.tile([128, TPB], f32, tag="var", bufs=2, name=f"var{b}")
                nc.vector.tensor_scalar_mul(MEAN, MU[:, bs:be], 1.0 / D)
                nc.vector.tensor_tensor(out=msq[:], in0=MEAN, in1=MEAN, op=ALU.mult)
                nc.vector.tensor_scalar(VAR[:], SQb, 1.0 / D, None, ALU.mult)
                nc.vector.tensor_tensor(out=VAR[:], in0=VAR[:], in1=msq[:], op=ALU.subtract)
                nc.vector.tensor_scalar_add(VAR[:], VAR[:], EPS_LN)
                STD = work.tile([128, TPB], f32, tag="std", bufs=2, name=f"std{b}")
                nc.scalar.activation(STD[:], VAR[:], AF.Sqrt)
                RSTD = work.tile([128, TPB], f32, tag="rstd", bufs=2, name=f"rstd{b}")
                nc.vector.reciprocal(RSTD[:], STD[:])
                for t in range(bs, be):
                    nc.vector.tensor_scalar(
                        OUT[:, t * 128:(t + 1) * 128], hblob[:, t * 128:(t + 1) * 128],
                        MEAN[:, t - bs:t - bs + 1], RSTD[:, t - bs:t - bs + 1],
                        ALU.subtract, ALU.mult)
                for t in range(bs, be):
                    sl = OUT[:, t * 128:(t + 1) * 128]
                    nc.gpsimd.tensor_tensor(out=sl, in0=sl, in1=GAM, op=ALU.mult)
                    nc.gpsimd.tensor_tensor(out=sl, in0=sl, in1=BET, op=ALU.add)
                nc.sync.dma_start(
                    yout[b * SEQ:(b + 1) * SEQ, :].rearrange("(t p) c -> p t c", p=128),
                    OUT[:, bs * 128:be * 128].rearrange("p (t c) -> p t c", c=128),
                )


            if debug:
                for nm, tl in [("d_xhat", xhat_t), ("d_y", y_t), ("d_v", v_r),
                               ("d_h", hblob), ("d_et", et[0]), ("d_ht", ht_sb),
                               ("d_zr", zrow)]:
                    dd = nc.dram_tensor(nm, list(tl.shape), f32,
                                        kind="ExternalOutput").ap()
                    if tl[:].dtype == f32r:
                        tmp = big.tile(list(tl.shape), f32, name=f"cv_{nm}")
                        nc.vector.tensor_copy(tmp[:], tl[:].bitcast(f32))
                        nc.gpsimd.dma_start(dd[:], tmp[:])
                    else:
                        nc.gpsimd.dma_start(dd[:], tl[:])
                dz = nc.dram_tensor("d_z", [128, TILES], f32, kind="ExternalOutput").ap()
                nc.gpsimd.dma_start(dz[:], zcols[:])
    nc.compile()
    return nc


def _get_nc():
    if "nc" not in _CACHE:
        _CACHE["nc"] = _build_program()
    return _CACHE["nc"]


def kernel(x, phi, Vw, Vb, gamma, beta):
    from concourse.bass_utils import run_bass_kernel_spmd

    x = np.asarray(x, dtype=np.float32)
    U = _build_U(np.asarray(phi)).astype(np.float32)

    cst = np.zeros((128, C_W), dtype=np.float32)
    cst[:, C_UT:C_UT + 128] = U.T
    cst[:, C_EYE:C_EYE + 128] = np.eye(128, dtype=np.float32)
    cst[:, C_VWT:C_VWT + 128] = np.asarray(Vw, np.float32).T
    cst[:, C_GAM:C_GAM + 128] = np.broadcast_to(np.asarray(gamma, np.float32), (128, 128))
    cst[:, C_BET:C_BET + 128] = np.broadcast_to(np.asarray(beta, np.float32), (128, 128))
    cst[:, C_VBR:C_VBR + 128] = np.broadcast_to(np.asarray(Vb, np.float32), (128, 128))
    cst[:, C_ONE] = 1.0

    nc = _get_nc()
    in_maps = []
    for c in range(N_CORES):
        xs = x[c * B_PER_CORE:(c + 1) * B_PER_CORE].reshape(B_PER_CORE * SEQ, D)
        in_maps.append({"xin": np.ascontiguousarray(xs), "cst": cst})
    res = run_bass_kernel_spmd(nc, in_maps, core_ids=list(range(N_CORES)))
    out = np.empty((BATCH, SEQ, D), dtype=np.float32)
    for c in range(N_CORES):
        out[c * B_PER_CORE:(c + 1) * B_PER_CORE] = (
            res.results[c]["yout"].reshape(B_PER_CORE, SEQ, D))
    return out


# revision 22
# speedup vs baseline: 1.0558x; 1.0261x over previous
"""nn_AttentionHead kernel for 8 Trainium2 NeuronCores.

Sharding: data-parallel over batch (16 batches -> 2 per core). phi/V/LN params
replicated; the [n,n] score matrix stays core-local.

Device pipeline per core (2 batches, n=1024, d=128), all heavy math on device:
  norms/xhat, Y = U @ xhat^T (scaled), scores^T = Y^T... (j,i)-layout,
  exp, rowsums via ones-matmul, attn@V in feature-major, transpose back,
  residual + LayerNorm.  Score/AV matmuls run in fp32r (TF32-class, 4x faster
  than fp32 on the PE); everything else fp32.

U(phi) is a fixed reparameterization of the phi parameter vector (8128 Givens
angles -> one orthogonal 128x128 matrix); it is prepared host-side in float64
and shipped as an input, like any other weight-layout preprocessing.
"""

import numpy as np

D = 128
SEQ = 1024
BATCH = 16
N_CORES = 8
B_PER_CORE = BATCH // N_CORES          # 2
TILES = B_PER_CORE * SEQ // 128        # 16 token tiles per core
TPB = SEQ // 128                       # 8 token tiles per batch
EPS_LN = 1e-5

# cst blob column layout
C_UT = 0          # U^T              [128, 128]
C_EYE = 128       # identity         [128, 128]
C_VWT = 256       # Vw^T             [128, 128]
C_GAM = 384       # gamma replicated [128, 128]
C_BET = 512       # beta replicated  [128, 128]
C_VBR = 640       # Vb replicated    [128, 128]
C_ONE = 768       # ones column      [128, 1]
C_W = 769


def _build_U(phi: np.ndarray) -> np.ndarray:
    d = D
    U = np.eye(d, dtype=np.float64)
    p = phi.astype(np.float64)
    k = 0
    for i in range(1, d):
        for j in range(i, 0, -1):
            a, b = j - 1, j
            c, s = np.cos(p[k]), np.sin(p[k])
            ra = U[a].copy()
            rb = U[b].copy()
            U[a] = c * ra + s * rb
            U[b] = -s * ra + c * rb
            k += 1
    return U


_CACHE = {}


def _build_program(debug=False):
    import concourse.bass as bass
    import concourse.tile as tile
    import concourse.mybir as mybir
    from concourse import bacc

    from concourse.dve_ops import AFFINE_THEN_ADD
    AF = mybir.ActivationFunctionType
    ALU = mybir.AluOpType
    f32 = mybir.dt.float32
    f32r = mybir.dt.float32r

    nc = bacc.Bacc(None, target_bir_lowering=False, num_devices=N_CORES)
    dbg = {}
    xin = nc.dram_tensor("xin", [B_PER_CORE * SEQ, D], f32, kind="ExternalInput").ap()
    cst = nc.dram_tensor("cst", [128, C_W], f32, kind="ExternalInput").ap()
    yout = nc.dram_tensor("yout", [B_PER_CORE * SEQ, D], f32, kind="ExternalOutput").ap()

    with tile.TileContext(nc) as tc:
        with (
            tc.tile_pool(name="big", bufs=1) as big,
            tc.tile_pool(name="work", bufs=2) as work,
            tc.tile_pool(name="ps512", bufs=2, space="PSUM") as ps512,
        ):
            # ---- loads ----
            cstt = big.tile([128, C_W], f32)
            nc.gpsimd.dma_start(cstt[:], cst[:])
            xrow = big.tile([128, TILES * 128], f32)  # token-major x, all tiles
            for bb in range(B_PER_CORE):
                nc.sync.dma_start(
                    xrow[:, bb * TPB * 128:(bb + 1) * TPB * 128]
                    .rearrange("p (t c) -> p t c", c=128),
                    xin[bb * SEQ:(bb + 1) * SEQ, :].rearrange("(t p) c -> p t c", p=128),
                )
            UT = cstt[:, C_UT:C_UT + 128]
            EYE = cstt[:, C_EYE:C_EYE + 128]
            VWT = cstt[:, C_VWT:C_VWT + 128]
            GAM = cstt[:, C_GAM:C_GAM + 128]
            BET = cstt[:, C_BET:C_BET + 128]
            VBR = cstt[:, C_VBR:C_VBR + 128]
            ONE = cstt[:, C_ONE:C_ONE + 1]

            ones_r = big.tile([128, 1], f32r)
            nc.vector.tensor_copy(ones_r[:], ONE)
            onesm = big.tile([128, 128], f32)
            nc.gpsimd.memset(onesm[:], 1.0)
            onesm_r = big.tile([128, 128], f32r)
            nc.vector.tensor_copy(onesm_r[:], onesm[:])
            vwt_r = big.tile([128, 128], f32r)
            nc.vector.tensor_copy(vwt_r[:], VWT)

            # ---- per-token stats: normsq via ACT Square accum_out ----
            NS = big.tile([128, TILES], f32)       # sum of squares per token
            for t in range(TILES):
                sc = work.tile([128, 128], f32, tag="sqs", name=f"sqs{t}")
                xt_ = xrow[:, t * 128:(t + 1) * 128]
                nc.vector.tensor_tensor(out=sc[:], in0=xt_, in1=xt_, op=ALU.mult)
                nc.vector.reduce_sum(NS[:, t:t + 1], sc[:],
                                     axis=mybir.AxisListType.X)
            norms = big.tile([128, TILES], f32)
            nc.scalar.activation(norms[:], NS[:], AF.Sqrt)
            normc = big.tile([128, TILES], f32)    # max(norms, eps)
            nc.vector.tensor_scalar_max(normc[:], norms[:], 1e-12)
            rinv = big.tile([128, TILES], f32)     # 1 / max(norms, eps)
            nc.vector.reciprocal(rinv[:], normc[:])
            rY = big.tile([128, TILES], f32)       # sqrt(rinv) * 128**-0.25
            nc.scalar.activation(rY[:], rinv[:], AF.Sqrt, scale=float(D ** -0.5))

            # ---- scaled rows + transposes ----
            xhat_t = big.tile([128, TILES * 128], f32r)   # xhat^T  (d-major)
            xs_t = big.tile([128, TILES * 128], f32)      # scaled  (d-major)
            for half, (dst, scl, dt_) in enumerate(
                [(xhat_t, rinv, f32r), (xs_t, rY, f32)]
            ):
                for q in range(TILES // 4):       # quads of 4 tiles
                    pt = ps512.tile([128, 512], f32, tag="q5", bufs=1, name=f"trq{half}_{q}")
                    for k in range(4):
                        t = q * 4 + k
                        row = work.tile([128, 128], f32, tag="rowsc",
                                        name=f"rs{half}_{t}")
                        nc.vector.tensor_scalar_mul(
                            row[:], xrow[:, t * 128:(t + 1) * 128], scl[:, t:t + 1])
                        nc.tensor.transpose(pt[:, k * 128:(k + 1) * 128], row[:], EYE)
                    nc.vector.tensor_copy(dst[:, q * 512:(q + 1) * 512], pt[:])

            # ---- Y^T = U @ xs^T  (fp32), evac to fp32r ----
            y_t = big.tile([128, TILES * 128], f32r)
            for q in range(TILES // 4):
                yp = ps512.tile([128, 512], f32, tag="q5", bufs=1, name=f"yq{q}")
                nc.tensor.matmul(yp[:], UT, xs_t[:, q * 512:(q + 1) * 512],
                                 start=True, stop=True)
                nc.vector.tensor_copy(y_t[:, q * 512:(q + 1) * 512], yp[:])

            # ---- V0 = x @ Vw^T  (token-major, fused norms scale on evac) ----
            v_r = big.tile([128, TILES * 128], f32r)
            for t in range(TILES):
                vp = ps512.tile([128, 128], f32, tag="sm", bufs=1, name=f"vp{t}")
                nc.tensor.matmul(vp[:], xhat_t[:, t * 128:(t + 1) * 128], vwt_r[:],
                                 start=True, stop=True)
                nc.vector.tensor_scalar_mul(v_r[:, t * 128:(t + 1) * 128], vp[:],
                                            norms[:, t:t + 1])

            # ---- XV = x + Vb (residual base) ----
            xv = big.tile([128, TILES * 128], f32)
            for t in range(TILES):
                nc.gpsimd.tensor_tensor(
                    out=xv[:, t * 128:(t + 1) * 128],
                    in0=xrow[:, t * 128:(t + 1) * 128], in1=VBR, op=ALU.add)

            OUT = big.tile([128, TILES * 128], f32)
            hblob = big.tile([128, TILES * 128], f32)
            MU = big.tile([128, TILES], f32)
            SQ = big.tile([128, TILES], f32)
            zcols = big.tile([128, TILES], f32)

            # ================= per-batch attention =================
            for b in range(B_PER_CORE):
                jbase = b * TPB
                et = [None] * TPB
                # scores^T tiles [j_tile, i] + exp
                for jt in range(TPB):
                    ett = work.tile([128, SEQ], f32r, tag="et", bufs=12,
                                    name=f"et{b}_{jt}")
                    et[jt] = ett
                    stp = ps512.tile([128, 1024], f32, tag="qst", bufs=1, name=f"st{b}_{jt}")
                    for ic in range(2):
                        nc.tensor.matmul(
                            stp[:, ic * 512:(ic + 1) * 512],
                            y_t[:, (jbase + jt) * 128:(jbase + jt + 1) * 128],
                            xhat_t[:, b * SEQ + ic * 512: b * SEQ + (ic + 1) * 512],
                            start=True, stop=True)
                    asb = work.tile([128, 1024], f32, tag="asb", bufs=4,
                                    name=f"asb{b}_{jt}")
                    nc.scalar.activation(asb[:], stp[:], AF.Square)
                    nc.scalar.activation(ett[:], asb[:], AF.Exp)
                # attn @ V (feature-major) + rowsums, accumulated over j tiles
                htp = [None, None]
                for ic in range(2):
                    htp[ic] = ps512.tile([128, 512], f32, tag=f"ht{ic}", bufs=1,
                                         name=f"ht{b}_{ic}")
                zzp = ps512.tile([128, 1024], f32, tag="zzp", bufs=1, name=f"zzp{b}")
                zp = [zzp[:, 0:512], zzp[:, 512:1024]]
                for jt in range(TPB):
                    for ic in range(2):
                        nc.tensor.matmul(
                            htp[ic][:], v_r[:, (jbase + jt) * 128:(jbase + jt + 1) * 128],
                            et[jt][:, ic * 512:(ic + 1) * 512],
                            start=(jt == 0), stop=(jt == TPB - 1))
                        nc.tensor.matmul(
                            zp[ic], onesm_r[:],
                            et[jt][:, ic * 512:(ic + 1) * 512],
                            start=(jt == 0), stop=(jt == TPB - 1))
                # evac Ht + Z
                ht_sb = work.tile([128, SEQ], f32, tag="htsb", bufs=2, name=f"htsb{b}")
                zrow = work.tile([1, SEQ], f32, tag="zrow", bufs=2, name=f"zrow{b}")
                for ic in range(2):
                    nc.scalar.copy(ht_sb[:, ic * 512:(ic + 1) * 512], htp[ic][:])
                nc.scalar.copy(zrow[:], zzp[:1, :])
                # redistribute Z row -> columns [128, TPB] via K=1 matmuls
                ztp = ps512.tile([128, TPB], f32, tag="sm", bufs=1, name=f"ztp{b}")
                for it in range(TPB):
                    nc.tensor.matmul(ztp[:, it:it + 1],
                                     zrow[:1, it * 128:(it + 1) * 128],
                                     ONE[:1, :1], start=(it == 0), stop=(it == TPB - 1))
                nc.scalar.copy(zcols[:, b * TPB:(b + 1) * TPB], ztp[:])
                # transpose Ht back to token-major, build h = Ht^T/Z + XV
                rz = work.tile([128, TPB], f32, tag="rz", bufs=2, name=f"rz{b}")
                nc.vector.reciprocal(rz[:], zcols[:, b * TPB:(b + 1) * TPB])
                for hq in range(2):
                    hp = ps512.tile([128, 512], f32, tag="q5", bufs=1, name=f"hq{b}_{hq}")
                    for k in range(4):
                        it = hq * 4 + k
                        nc.tensor.transpose(hp[:, k * 128:(k + 1) * 128],
                                            ht_sb[:, it * 128:(it + 1) * 128], EYE)
                    for k in range(4):
                        it = hq * 4 + k
                        t = b * TPB + it
                        nc.vector._custom_dve(
                            AFFINE_THEN_ADD,
                            out=hblob[:, t * 128:(t + 1) * 128],
                            in0=hp[:, k * 128:(k + 1) * 128],
                            in1=xv[:, t * 128:(t + 1) * 128],
                            s0=rz[:, it:it + 1], s1=0.0)

                # ---- LayerNorm for this batch's 8 tiles ----
                bs, be = b * TPB, (b + 1) * TPB
                for t in range(bs, be):
                    hs = hblob[:, t * 128:(t + 1) * 128]
                    nc.vector.reduce_sum(MU[:, t:t + 1], hs, axis=mybir.AxisListType.X)
                    sc2 = work.tile([128, 128], f32, tag="sqs2", bufs=2, name=f"sq2_{t}")
                    nc.vector.tensor_tensor(out=sc2[:], in0=hs, in1=hs, op=ALU.mult)
                    nc.vector.reduce_sum(SQ[:, t:t + 1], sc2[:], axis=mybir.AxisListType.X)
                MEAN = MU[:, bs:be]
                SQb = SQ[:, bs:be]
                msq = work.tile([128, TPB], f32, tag="msq", bufs=2, name=f"msq{b}")
                VAR = work.tile([128, TPB], f32, tag="var", bufs=2, name=f"var{b}")
                nc.vector.tensor_scalar_mul(MEAN, MU[:, bs:be], 1.0 / D)
                nc.vector.tensor_tensor(out=msq[:], in0=MEAN, in1=MEAN, op=ALU.mult)
                nc.vector.tensor_scalar(VAR[:], SQb, 1.0 / D, None, ALU.mult)
                nc.vector.tensor_tensor(out=VAR[:], in0=VAR[:], in1=msq[:], op=ALU.subtract)
                nc.vector.tensor_scalar_add(VAR[:], VAR[:], EPS_LN)
                STD = work.tile([128, TPB], f32, tag="std", bufs=2, name=f"std{b}")
                nc.scalar.activation(STD[:], VAR[:], AF.Sqrt)
                RSTD = work.tile([128, TPB], f32, tag="rstd", bufs=2, name=f"rstd{b}")
                nc.vector.reciprocal(RSTD[:], STD[:])
                for t in range(bs, be):
                    nc.vector.tensor_scalar(
                        OUT[:, t * 128:(t + 1) * 128], hblob[:, t * 128:(t + 1) * 128],
                        MEAN[:, t - bs:t - bs + 1], RSTD[:, t - bs:t - bs + 1],
                        ALU.subtract, ALU.mult)
                for t in range(bs, be):
                    sl = OUT[:, t * 128:(t + 1) * 128]
                    nc.gpsimd.tensor_tensor(out=sl, in0=sl, in1=GAM, op=ALU.mult)
                    nc.gpsimd.tensor_tensor(out=sl, in0=sl, in1=BET, op=ALU.add)
                nc.sync.dma_start(
                    yout[b * SEQ:(b + 1) * SEQ, :].rearrange("(t p) c -> p t c", p=128),
                    OUT[:, bs * 128:be * 128].rearrange("p (t c) -> p t c", c=128),
                )


            if debug:
                for nm, tl in [("d_xhat", xhat_t), ("d_y", y_t), ("d_v", v_r),
                               ("d_h", hblob), ("d_et", et[0]), ("d_ht", ht_sb),
                               ("d_zr", zrow)]:
                    dd = nc.dram_tensor(nm, list(tl.shape), f32,
                                        kind="ExternalOutput").ap()
                    if tl[:].dtype == f32r:
                        tmp = big.tile(list(tl.shape), f32, name=f"cv_{nm}")
                        nc.vector.tensor_copy(tmp[:], tl[:].bitcast(f32))
                        nc.gpsimd.dma_start(dd[:], tmp[:])
                    else:
                        nc.gpsimd.dma_start(dd[:], tl[:])
                dz = nc.dram_tensor("d_z", [128, TILES], f32, kind="ExternalOutput").ap()
                nc.gpsimd.dma_start(dz[:], zcols[:])
    nc.compile()
    return nc


def _get_nc():
    if "nc" not in _CACHE:
        _CACHE["nc"] = _build_program()
    return _CACHE["nc"]


def kernel(x, phi, Vw, Vb, gamma, beta):
    from concourse.bass_utils import run_bass_kernel_spmd

    x = np.asarray(x, dtype=np.float32)
    U = _build_U(np.asarray(phi)).astype(np.float32)

    cst = np.zeros((128, C_W), dtype=np.float32)
    cst[:, C_UT:C_UT + 128] = U.T
    cst[:, C_EYE:C_EYE + 128] = np.eye(128, dtype=np.float32)
    cst[:, C_VWT:C_VWT + 128] = np.asarray(Vw, np.float32).T
    cst[:, C_GAM:C_GAM + 128] = np.broadcast_to(np.asarray(gamma, np.float32), (128, 128))
    cst[:, C_BET:C_BET + 128] = np.broadcast_to(np.asarray(beta, np.float32), (128, 128))
    cst[:, C_VBR:C_VBR + 128] = np.broadcast_to(np.asarray(Vb, np.float32), (128, 128))
    cst[:, C_ONE] = 1.0

    nc = _get_nc()
    in_maps = []
    for c in range(N_CORES):
        xs = x[c * B_PER_CORE:(c + 1) * B_PER_CORE].reshape(B_PER_CORE * SEQ, D)
        in_maps.append({"xin": np.ascontiguousarray(xs), "cst": cst})
    res = run_bass_kernel_spmd(nc, in_maps, core_ids=list(range(N_CORES)))
    out = np.empty((BATCH, SEQ, D), dtype=np.float32)
    for c in range(N_CORES):
        out[c * B_PER_CORE:(c + 1) * B_PER_CORE] = (
            res.results[c]["yout"].reshape(B_PER_CORE, SEQ, D))
    return out


# revision 23
# speedup vs baseline: 1.0778x; 1.0208x over previous
"""nn_AttentionHead kernel for 8 Trainium2 NeuronCores.

Sharding: data-parallel over batch (16 batches -> 2 per core). phi/V/LN params
replicated; the [n,n] score matrix stays core-local.

Device pipeline per core (2 batches, n=1024, d=128), all heavy math on device:
  norms/xhat, Y = U @ xhat^T (scaled), scores^T = Y^T... (j,i)-layout,
  exp, rowsums via ones-matmul, attn@V in feature-major, transpose back,
  residual + LayerNorm.  Score/AV matmuls run in fp32r (TF32-class, 4x faster
  than fp32 on the PE); everything else fp32.

U(phi) is a fixed reparameterization of the phi parameter vector (8128 Givens
angles -> one orthogonal 128x128 matrix); it is prepared host-side in float64
and shipped as an input, like any other weight-layout preprocessing.
"""

import numpy as np

D = 128
SEQ = 1024
BATCH = 16
N_CORES = 8
B_PER_CORE = BATCH // N_CORES          # 2
TILES = B_PER_CORE * SEQ // 128        # 16 token tiles per core
TPB = SEQ // 128                       # 8 token tiles per batch
EPS_LN = 1e-5

# cst blob column layout
C_UT = 0          # U^T              [128, 128]
C_EYE = 128       # identity         [128, 128]
C_VWT = 256       # Vw^T             [128, 128]
C_GAM = 384       # gamma replicated [128, 128]
C_BET = 512       # beta replicated  [128, 128]
C_VBR = 640       # Vb replicated    [128, 128]
C_ONE = 768       # ones column      [128, 1]
C_W = 769


def _build_U(phi: np.ndarray) -> np.ndarray:
    d = D
    U = np.eye(d, dtype=np.float64)
    p = phi.astype(np.float64)
    k = 0
    for i in range(1, d):
        for j in range(i, 0, -1):
            a, b = j - 1, j
            c, s = np.cos(p[k]), np.sin(p[k])
            ra = U[a].copy()
            rb = U[b].copy()
            U[a] = c * ra + s * rb
            U[b] = -s * ra + c * rb
            k += 1
    return U


_CACHE = {}


def _build_program(debug=False):
    import concourse.bass as bass
    import concourse.tile as tile
    import concourse.mybir as mybir
    from concourse import bacc

    from concourse.dve_ops import AFFINE_THEN_ADD
    AF = mybir.ActivationFunctionType
    ALU = mybir.AluOpType
    f32 = mybir.dt.float32
    f32r = mybir.dt.float32r

    nc = bacc.Bacc(None, target_bir_lowering=False, num_devices=N_CORES)
    dbg = {}
    xin = nc.dram_tensor("xin", [B_PER_CORE * SEQ, D], f32, kind="ExternalInput").ap()
    cst = nc.dram_tensor("cst", [128, C_W], f32, kind="ExternalInput").ap()
    yout = nc.dram_tensor("yout", [B_PER_CORE * SEQ, D], f32, kind="ExternalOutput").ap()

    with tile.TileContext(nc) as tc:
        with (
            tc.tile_pool(name="big", bufs=1) as big,
            tc.tile_pool(name="work", bufs=2) as work,
            tc.tile_pool(name="ps512", bufs=2, space="PSUM") as ps512,
        ):
            # ---- loads ----
            cstt = big.tile([128, C_W], f32)
            nc.gpsimd.dma_start(cstt[:], cst[:])
            xrow = big.tile([128, TILES * 128], f32)  # token-major x, all tiles
            for bb in range(B_PER_CORE):
                nc.sync.dma_start(
                    xrow[:, bb * TPB * 128:(bb + 1) * TPB * 128]
                    .rearrange("p (t c) -> p t c", c=128),
                    xin[bb * SEQ:(bb + 1) * SEQ, :].rearrange("(t p) c -> p t c", p=128),
                )
            UT = cstt[:, C_UT:C_UT + 128]
            EYE = cstt[:, C_EYE:C_EYE + 128]
            VWT = cstt[:, C_VWT:C_VWT + 128]
            GAM = cstt[:, C_GAM:C_GAM + 128]
            BET = cstt[:, C_BET:C_BET + 128]
            VBR = cstt[:, C_VBR:C_VBR + 128]
            ONE = cstt[:, C_ONE:C_ONE + 1]

            ones_r = big.tile([128, 1], f32r)
            nc.vector.tensor_copy(ones_r[:], ONE)
            onesm = big.tile([128, 128], f32)
            nc.gpsimd.memset(onesm[:], 1.0)
            onesm_r = big.tile([128, 128], f32r)
            nc.vector.tensor_copy(onesm_r[:], onesm[:])
            vwt_r = big.tile([128, 128], f32r)
            nc.vector.tensor_copy(vwt_r[:], VWT)

            # ---- per-token stats: normsq via ACT Square accum_out ----
            NS = big.tile([128, TILES], f32)       # sum of squares per token
            for t in range(TILES):
                sc = work.tile([128, 128], f32, tag="sqs", name=f"sqs{t}")
                xt_ = xrow[:, t * 128:(t + 1) * 128]
                nc.vector.tensor_tensor(out=sc[:], in0=xt_, in1=xt_, op=ALU.mult)
                nc.vector.reduce_sum(NS[:, t:t + 1], sc[:],
                                     axis=mybir.AxisListType.X)
            norms = big.tile([128, TILES], f32)
            nc.scalar.activation(norms[:], NS[:], AF.Sqrt)
            normc = big.tile([128, TILES], f32)    # max(norms, eps)
            nc.vector.tensor_scalar_max(normc[:], norms[:], 1e-12)
            rinv = big.tile([128, TILES], f32)     # 1 / max(norms, eps)
            nc.vector.reciprocal(rinv[:], normc[:])
            rY = big.tile([128, TILES], f32)       # sqrt(rinv) * 128**-0.25
            nc.scalar.activation(rY[:], rinv[:], AF.Sqrt, scale=float(D ** -0.5))

            # ---- scaled rows + transposes ----
            xhat_t = big.tile([128, TILES * 128], f32r)   # xhat^T  (d-major)
            xs_t = big.tile([128, TILES * 128], f32)      # scaled  (d-major)
            for half, (dst, scl, dt_) in enumerate(
                [(xhat_t, rinv, f32r), (xs_t, rY, f32)]
            ):
                for q in range(TILES // 4):       # quads of 4 tiles
                    pt = ps512.tile([128, 512], f32, tag="q5", bufs=1, name=f"trq{half}_{q}")
                    for k in range(4):
                        t = q * 4 + k
                        row = work.tile([128, 128], f32, tag="rowsc",
                                        name=f"rs{half}_{t}")
                        nc.vector.tensor_scalar_mul(
                            row[:], xrow[:, t * 128:(t + 1) * 128], scl[:, t:t + 1])
                        nc.tensor.transpose(pt[:, k * 128:(k + 1) * 128], row[:], EYE)
                    nc.vector.tensor_copy(dst[:, q * 512:(q + 1) * 512], pt[:])

            # ---- Y^T = U @ xs^T  (fp32), evac to fp32r ----
            y_t = big.tile([128, TILES * 128], f32r)
            for q in range(TILES // 4):
                yp = ps512.tile([128, 512], f32, tag="q5", bufs=1, name=f"yq{q}")
                nc.tensor.matmul(yp[:], UT, xs_t[:, q * 512:(q + 1) * 512],
                                 start=True, stop=True)
                nc.vector.tensor_copy(y_t[:, q * 512:(q + 1) * 512], yp[:])

            # ---- V0 = x @ Vw^T  (token-major, fused norms scale on evac) ----
            v_r = big.tile([128, TILES * 128], f32r)
            for t in range(TILES):
                vp = ps512.tile([128, 128], f32, tag="sm", bufs=1, name=f"vp{t}")
                nc.tensor.matmul(vp[:], xhat_t[:, t * 128:(t + 1) * 128], vwt_r[:],
                                 start=True, stop=True)
                nc.vector.tensor_scalar_mul(v_r[:, t * 128:(t + 1) * 128], vp[:],
                                            norms[:, t:t + 1])

            # ---- XV = x + Vb (residual base) ----
            xv = big.tile([128, TILES * 128], f32)
            for t in range(TILES):
                nc.gpsimd.tensor_tensor(
                    out=xv[:, t * 128:(t + 1) * 128],
                    in0=xrow[:, t * 128:(t + 1) * 128], in1=VBR, op=ALU.add)

            OUT = big.tile([128, TILES * 128], f32)
            hblob = big.tile([128, TILES * 128], f32)
            MU = big.tile([128, TILES], f32)
            SQ = big.tile([128, TILES], f32)
            zcols = big.tile([128, TILES], f32)

            # ================= per-batch attention =================
            for b in range(B_PER_CORE):
                jbase = b * TPB
                et = [None] * TPB
                # scores^T tiles [j_tile, i] + exp
                for jt in range(TPB):
                    ett = work.tile([128, SEQ], f32r, tag="et", bufs=12,
                                    name=f"et{b}_{jt}")
                    et[jt] = ett
                    stp = ps512.tile([128, 1024], f32, tag="qst", bufs=1, name=f"st{b}_{jt}")
                    for ic in range(2):
                        nc.tensor.matmul(
                            stp[:, ic * 512:(ic + 1) * 512],
                            y_t[:, (jbase + jt) * 128:(jbase + jt + 1) * 128],
                            xhat_t[:, b * SEQ + ic * 512: b * SEQ + (ic + 1) * 512],
                            start=True, stop=True)
                    asb = work.tile([128, 1024], f32, tag="asb", bufs=4,
                                    name=f"asb{b}_{jt}")
                    nc.scalar.activation(asb[:], stp[:], AF.Square)
                    nc.scalar.activation(ett[:], asb[:], AF.Exp)
                # attn @ V (feature-major) + rowsums, accumulated over j tiles
                htp = [None, None]
                for ic in range(2):
                    htp[ic] = ps512.tile([128, 512], f32, tag=f"ht{ic}", bufs=1,
                                         name=f"ht{b}_{ic}")
                zzp = ps512.tile([128, 1024], f32, tag="zzp", bufs=1, name=f"zzp{b}")
                zp = [zzp[:, 0:512], zzp[:, 512:1024]]
                for jt in range(TPB):
                    for ic in range(2):
                        nc.tensor.matmul(
                            htp[ic][:], v_r[:, (jbase + jt) * 128:(jbase + jt + 1) * 128],
                            et[jt][:, ic * 512:(ic + 1) * 512],
                            start=(jt == 0), stop=(jt == TPB - 1))
                        nc.tensor.matmul(
                            zp[ic], onesm_r[:],
                            et[jt][:, ic * 512:(ic + 1) * 512],
                            start=(jt == 0), stop=(jt == TPB - 1))
                # evac Ht + Z
                ht_sb = work.tile([128, SEQ], f32, tag="htsb", bufs=2, name=f"htsb{b}")
                zrow = work.tile([1, SEQ], f32, tag="zrow", bufs=2, name=f"zrow{b}")
                for ic in range(2):
                    nc.vector.tensor_copy(ht_sb[:, ic * 512:(ic + 1) * 512], htp[ic][:])
                nc.vector.tensor_copy(zrow[:], zzp[:1, :])
                # redistribute Z row -> columns [128, TPB] via K=1 matmuls
                ztp = ps512.tile([128, TPB], f32, tag="sm", bufs=1, name=f"ztp{b}")
                for it in range(TPB):
                    nc.tensor.matmul(ztp[:, it:it + 1],
                                     zrow[:1, it * 128:(it + 1) * 128],
                                     ONE[:1, :1], start=(it == 0), stop=(it == TPB - 1))
                nc.scalar.copy(zcols[:, b * TPB:(b + 1) * TPB], ztp[:])
                # transpose Ht back to token-major, build h = Ht^T/Z + XV
                rz = work.tile([128, TPB], f32, tag="rz", bufs=2, name=f"rz{b}")
                nc.vector.reciprocal(rz[:], zcols[:, b * TPB:(b + 1) * TPB])
                for hq in range(2):
                    hp = ps512.tile([128, 512], f32, tag="q5", bufs=1, name=f"hq{b}_{hq}")
                    for k in range(4):
                        it = hq * 4 + k
                        nc.tensor.transpose(hp[:, k * 128:(k + 1) * 128],
                                            ht_sb[:, it * 128:(it + 1) * 128], EYE)
                    for k in range(4):
                        it = hq * 4 + k
                        t = b * TPB + it
                        nc.vector._custom_dve(
                            AFFINE_THEN_ADD,
                            out=hblob[:, t * 128:(t + 1) * 128],
                            in0=hp[:, k * 128:(k + 1) * 128],
                            in1=xv[:, t * 128:(t + 1) * 128],
                            s0=rz[:, it:it + 1], s1=0.0)

                # ---- LayerNorm for this batch's 8 tiles ----
                bs, be = b * TPB, (b + 1) * TPB
                for t in range(bs, be):
                    hs = hblob[:, t * 128:(t + 1) * 128]
                    nc.vector.reduce_sum(MU[:, t:t + 1], hs, axis=mybir.AxisListType.X)
                    sc2 = work.tile([128, 128], f32, tag="sqs2", bufs=2, name=f"sq2_{t}")
                    nc.vector.tensor_tensor(out=sc2[:], in0=hs, in1=hs, op=ALU.mult)
                    nc.vector.reduce_sum(SQ[:, t:t + 1], sc2[:], axis=mybir.AxisListType.X)
                MEAN = MU[:, bs:be]
                SQb = SQ[:, bs:be]
                msq = work.tile([128, TPB], f32, tag="msq", bufs=2, name=f"msq{b}")
                VAR = work.tile([128, TPB], f32, tag="var", bufs=2, name=f"var{b}")
                nc.vector.tensor_scalar_mul(MEAN, MU[:, bs:be], 1.0 / D)
                nc.vector.tensor_tensor(out=msq[:], in0=MEAN, in1=MEAN, op=ALU.mult)
                nc.vector.tensor_scalar(VAR[:], SQb, 1.0 / D, None, ALU.mult)
                nc.vector.tensor_tensor(out=VAR[:], in0=VAR[:], in1=msq[:], op=ALU.subtract)
                nc.vector.tensor_scalar_add(VAR[:], VAR[:], EPS_LN)
                STD = work.tile([128, TPB], f32, tag="std", bufs=2, name=f"std{b}")
                nc.scalar.activation(STD[:], VAR[:], AF.Sqrt)
                RSTD = work.tile([128, TPB], f32, tag="rstd", bufs=2, name=f"rstd{b}")
                nc.vector.reciprocal(RSTD[:], STD[:])
                for t in range(bs, be):
                    nc.vector.tensor_scalar(
                        OUT[:, t * 128:(t + 1) * 128], hblob[:, t * 128:(t + 1) * 128],
                        MEAN[:, t - bs:t - bs + 1], RSTD[:, t - bs:t - bs + 1],
                        ALU.subtract, ALU.mult)
                for t in range(bs, be):
                    sl = OUT[:, t * 128:(t + 1) * 128]
                    nc.gpsimd.tensor_tensor(out=sl, in0=sl, in1=GAM, op=ALU.mult)
                    nc.gpsimd.tensor_tensor(out=sl, in0=sl, in1=BET, op=ALU.add)
                nc.sync.dma_start(
                    yout[b * SEQ:(b + 1) * SEQ, :].rearrange("(t p) c -> p t c", p=128),
                    OUT[:, bs * 128:be * 128].rearrange("p (t c) -> p t c", c=128),
                )


            if debug:
                for nm, tl in [("d_xhat", xhat_t), ("d_y", y_t), ("d_v", v_r),
                               ("d_h", hblob), ("d_et", et[0]), ("d_ht", ht_sb),
                               ("d_zr", zrow)]:
                    dd = nc.dram_tensor(nm, list(tl.shape), f32,
                                        kind="ExternalOutput").ap()
                    if tl[:].dtype == f32r:
                        tmp = big.tile(list(tl.shape), f32, name=f"cv_{nm}")
                        nc.vector.tensor_copy(tmp[:], tl[:].bitcast(f32))
                        nc.gpsimd.dma_start(dd[:], tmp[:])
                    else:
                        nc.gpsimd.dma_start(dd[:], tl[:])
                dz = nc.dram_tensor("d_z", [128, TILES], f32, kind="ExternalOutput").ap()
                nc.gpsimd.dma_start(dz[:], zcols[:])
    nc.compile()
    return nc


def _get_nc():
    if "nc" not in _CACHE:
        _CACHE["nc"] = _build_program()
    return _CACHE["nc"]


def kernel(x, phi, Vw, Vb, gamma, beta):
    from concourse.bass_utils import run_bass_kernel_spmd

    x = np.asarray(x, dtype=np.float32)
    U = _build_U(np.asarray(phi)).astype(np.float32)

    cst = np.zeros((128, C_W), dtype=np.float32)
    cst[:, C_UT:C_UT + 128] = U.T
    cst[:, C_EYE:C_EYE + 128] = np.eye(128, dtype=np.float32)
    cst[:, C_VWT:C_VWT + 128] = np.asarray(Vw, np.float32).T
    cst[:, C_GAM:C_GAM + 128] = np.broadcast_to(np.asarray(gamma, np.float32), (128, 128))
    cst[:, C_BET:C_BET + 128] = np.broadcast_to(np.asarray(beta, np.float32), (128, 128))
    cst[:, C_VBR:C_VBR + 128] = np.broadcast_to(np.asarray(Vb, np.float32), (128, 128))
    cst[:, C_ONE] = 1.0

    nc = _get_nc()
    in_maps = []
    for c in range(N_CORES):
        xs = x[c * B_PER_CORE:(c + 1) * B_PER_CORE].reshape(B_PER_CORE * SEQ, D)
        in_maps.append({"xin": np.ascontiguousarray(xs), "cst": cst})
    res = run_bass_kernel_spmd(nc, in_maps, core_ids=list(range(N_CORES)))
    out = np.empty((BATCH, SEQ, D), dtype=np.float32)
    for c in range(N_CORES):
        out[c * B_PER_CORE:(c + 1) * B_PER_CORE] = (
            res.results[c]["yout"].reshape(B_PER_CORE, SEQ, D))
    return out


# revision 24
# speedup vs baseline: 1.1011x; 1.0217x over previous
"""nn_AttentionHead kernel for 8 Trainium2 NeuronCores.

Sharding: data-parallel over batch (16 batches -> 2 per core). phi/V/LN params
replicated; the [n,n] score matrix stays core-local.

Device pipeline per core (2 batches, n=1024, d=128), all heavy math on device:
  norms/xhat, Y = U @ xhat^T (scaled), scores^T = Y^T... (j,i)-layout,
  exp, rowsums via ones-matmul, attn@V in feature-major, transpose back,
  residual + LayerNorm.  Score/AV matmuls run in fp32r (TF32-class, 4x faster
  than fp32 on the PE); everything else fp32.

U(phi) is a fixed reparameterization of the phi parameter vector (8128 Givens
angles -> one orthogonal 128x128 matrix); it is prepared host-side in float64
and shipped as an input, like any other weight-layout preprocessing.
"""

import numpy as np

D = 128
SEQ = 1024
BATCH = 16
N_CORES = 8
B_PER_CORE = BATCH // N_CORES          # 2
TILES = B_PER_CORE * SEQ // 128        # 16 token tiles per core
TPB = SEQ // 128                       # 8 token tiles per batch
EPS_LN = 1e-5

# cst blob column layout
C_UT = 0          # U^T              [128, 128]
C_EYE = 128       # identity         [128, 128]
C_VWT = 256       # Vw^T             [128, 128]
C_GAM = 384       # gamma replicated [128, 128]
C_BET = 512       # beta replicated  [128, 128]
C_VBR = 640       # Vb replicated    [128, 128]
C_ONE = 768       # ones column      [128, 1]
C_W = 769


def _build_U(phi: np.ndarray) -> np.ndarray:
    d = D
    U = np.eye(d, dtype=np.float64)
    p = phi.astype(np.float64)
    k = 0
    for i in range(1, d):
        for j in range(i, 0, -1):
            a, b = j - 1, j
            c, s = np.cos(p[k]), np.sin(p[k])
            ra = U[a].copy()
            rb = U[b].copy()
            U[a] = c * ra + s * rb
            U[b] = -s * ra + c * rb
            k += 1
    return U


_CACHE = {}


def _build_program(debug=False):
    import concourse.bass as bass
    import concourse.tile as tile
    import concourse.mybir as mybir
    from concourse import bacc

    from concourse.dve_ops import AFFINE_THEN_ADD
    AF = mybir.ActivationFunctionType
    ALU = mybir.AluOpType
    f32 = mybir.dt.float32
    f32r = mybir.dt.float32r

    nc = bacc.Bacc(None, target_bir_lowering=False, num_devices=N_CORES)
    dbg = {}
    xin = nc.dram_tensor("xin", [B_PER_CORE * SEQ, D], f32, kind="ExternalInput").ap()
    cst = nc.dram_tensor("cst", [128, C_W], f32, kind="ExternalInput").ap()
    yout = nc.dram_tensor("yout", [B_PER_CORE * SEQ, D], f32, kind="ExternalOutput").ap()

    with tile.TileContext(nc) as tc:
        with (
            tc.tile_pool(name="big", bufs=1) as big,
            tc.tile_pool(name="work", bufs=2) as work,
            tc.tile_pool(name="ps512", bufs=2, space="PSUM") as ps512,
        ):
            # ---- loads ----
            cstt = big.tile([128, C_W], f32)
            nc.gpsimd.dma_start(cstt[:], cst[:])
            xrow = big.tile([128, TILES * 128], f32)  # token-major x, all tiles
            for bb in range(B_PER_CORE):
                nc.sync.dma_start(
                    xrow[:, bb * TPB * 128:(bb + 1) * TPB * 128]
                    .rearrange("p (t c) -> p t c", c=128),
                    xin[bb * SEQ:(bb + 1) * SEQ, :].rearrange("(t p) c -> p t c", p=128),
                )
            UT = cstt[:, C_UT:C_UT + 128]
            EYE = cstt[:, C_EYE:C_EYE + 128]
            VWT = cstt[:, C_VWT:C_VWT + 128]
            GAM = cstt[:, C_GAM:C_GAM + 128]
            BET = cstt[:, C_BET:C_BET + 128]
            VBR = cstt[:, C_VBR:C_VBR + 128]
            ONE = cstt[:, C_ONE:C_ONE + 1]

            ones_r = big.tile([128, 1], f32r)
            nc.vector.tensor_copy(ones_r[:], ONE)
            onesm = big.tile([128, 128], f32)
            nc.gpsimd.memset(onesm[:], 1.0)
            onesm_r = big.tile([128, 128], f32r)
            nc.vector.tensor_copy(onesm_r[:], onesm[:])
            vwt_r = big.tile([128, 128], f32r)
            nc.vector.tensor_copy(vwt_r[:], VWT)

            # ---- per-token stats: normsq via ACT Square accum_out ----
            NS = big.tile([128, TILES], f32)       # sum of squares per token
            for t in range(TILES):
                sc = work.tile([128, 128], f32, tag="sqs", name=f"sqs{t}")
                xt_ = xrow[:, t * 128:(t + 1) * 128]
                nc.vector.tensor_tensor(out=sc[:], in0=xt_, in1=xt_, op=ALU.mult)
                nc.vector.reduce_sum(NS[:, t:t + 1], sc[:],
                                     axis=mybir.AxisListType.X)
            norms = big.tile([128, TILES], f32)
            nc.scalar.activation(norms[:], NS[:], AF.Sqrt)
            normc = big.tile([128, TILES], f32)    # max(norms, eps)
            nc.vector.tensor_scalar_max(normc[:], norms[:], 1e-12)
            rinv = big.tile([128, TILES], f32)     # 1 / max(norms, eps)
            nc.vector.reciprocal(rinv[:], normc[:])
            rY = big.tile([128, TILES], f32)       # sqrt(rinv) * 128**-0.25
            nc.scalar.activation(rY[:], rinv[:], AF.Sqrt, scale=float(D ** -0.5))

            # ---- scaled rows + transposes ----
            ut_r = big.tile([128, 128], f32r)
            nc.vector.tensor_copy(ut_r[:], UT)
            xhat_t = big.tile([128, TILES * 128], f32r)   # xhat^T  (d-major)
            xs_t = big.tile([128, TILES * 128], f32r)     # scaled  (d-major)
            for half, (dst, scl, dt_) in enumerate(
                [(xhat_t, rinv, f32r), (xs_t, rY, f32r)]
            ):
                for q in range(TILES // 4):       # quads of 4 tiles
                    pt = ps512.tile([128, 512], f32, tag="q5", bufs=1, name=f"trq{half}_{q}")
                    for k in range(4):
                        t = q * 4 + k
                        row = work.tile([128, 128], f32, tag="rowsc",
                                        name=f"rs{half}_{t}")
                        nc.vector.tensor_scalar_mul(
                            row[:], xrow[:, t * 128:(t + 1) * 128], scl[:, t:t + 1])
                        nc.tensor.transpose(pt[:, k * 128:(k + 1) * 128], row[:], EYE)
                    nc.vector.tensor_copy(dst[:, q * 512:(q + 1) * 512], pt[:])

            # ---- Y^T = U @ xs^T  (fp32), evac to fp32r ----
            y_t = big.tile([128, TILES * 128], f32r)
            for q in range(TILES // 4):
                yp = ps512.tile([128, 512], f32, tag="q5", bufs=1, name=f"yq{q}")
                nc.tensor.matmul(yp[:], ut_r[:], xs_t[:, q * 512:(q + 1) * 512],
                                 start=True, stop=True)
                nc.vector.tensor_copy(y_t[:, q * 512:(q + 1) * 512], yp[:])

            # ---- V0 = x @ Vw^T  (token-major, fused norms scale on evac) ----
            v_r = big.tile([128, TILES * 128], f32r)
            for q in range(TILES // 4):
                vp = ps512.tile([128, 512], f32, tag="sm", bufs=1, name=f"vp{q}")
                for k in range(4):
                    t = q * 4 + k
                    nc.tensor.matmul(vp[:, k * 128:(k + 1) * 128],
                                     xhat_t[:, t * 128:(t + 1) * 128], vwt_r[:],
                                     start=True, stop=True)
                for k in range(4):
                    t = q * 4 + k
                    nc.vector.tensor_scalar_mul(
                        v_r[:, t * 128:(t + 1) * 128], vp[:, k * 128:(k + 1) * 128],
                        norms[:, t:t + 1])

            # ---- XV = x + Vb (residual base) ----
            xv = big.tile([128, TILES * 128], f32)
            for t in range(TILES):
                nc.gpsimd.tensor_tensor(
                    out=xv[:, t * 128:(t + 1) * 128],
                    in0=xrow[:, t * 128:(t + 1) * 128], in1=VBR, op=ALU.add)

            OUT = big.tile([128, TILES * 128], f32)
            hblob = big.tile([128, TILES * 128], f32)
            MU = big.tile([128, TILES], f32)
            SQ = big.tile([128, TILES], f32)
            zcols = big.tile([128, TILES], f32)

            # ================= per-batch attention =================
            for b in range(B_PER_CORE):
                jbase = b * TPB
                et = [None] * TPB
                # scores^T tiles [j_tile, i] + exp
                for jt in range(TPB):
                    ett = work.tile([128, SEQ], f32r, tag="et", bufs=12,
                                    name=f"et{b}_{jt}")
                    et[jt] = ett
                    stp = ps512.tile([128, 1024], f32, tag="qst", bufs=1, name=f"st{b}_{jt}")
                    for ic in range(2):
                        nc.tensor.matmul(
                            stp[:, ic * 512:(ic + 1) * 512],
                            y_t[:, (jbase + jt) * 128:(jbase + jt + 1) * 128],
                            xhat_t[:, b * SEQ + ic * 512: b * SEQ + (ic + 1) * 512],
                            start=True, stop=True)
                    asb = work.tile([128, 1024], f32, tag="asb", bufs=4,
                                    name=f"asb{b}_{jt}")
                    nc.scalar.activation(asb[:], stp[:], AF.Square)
                    nc.scalar.activation(ett[:], asb[:], AF.Exp)
                # attn @ V (feature-major) + rowsums, accumulated over j tiles
                htp = [None, None]
                for ic in range(2):
                    htp[ic] = ps512.tile([128, 512], f32, tag=f"ht{ic}", bufs=1,
                                         name=f"ht{b}_{ic}")
                zzp = ps512.tile([128, 1024], f32, tag="zzp", bufs=1, name=f"zzp{b}")
                zp = [zzp[:, 0:512], zzp[:, 512:1024]]
                for jt in range(TPB):
                    for ic in range(2):
                        nc.tensor.matmul(
                            htp[ic][:], v_r[:, (jbase + jt) * 128:(jbase + jt + 1) * 128],
                            et[jt][:, ic * 512:(ic + 1) * 512],
                            start=(jt == 0), stop=(jt == TPB - 1))
                        nc.tensor.matmul(
                            zp[ic], onesm_r[:],
                            et[jt][:, ic * 512:(ic + 1) * 512],
                            start=(jt == 0), stop=(jt == TPB - 1))
                # evac Ht + Z
                ht_sb = work.tile([128, SEQ], f32, tag="htsb", bufs=2, name=f"htsb{b}")
                zrow = work.tile([1, SEQ], f32, tag="zrow", bufs=2, name=f"zrow{b}")
                for ic in range(2):
                    nc.vector.tensor_copy(ht_sb[:, ic * 512:(ic + 1) * 512], htp[ic][:])
                nc.vector.tensor_copy(zrow[:], zzp[:1, :])
                # redistribute Z row -> columns [128, TPB] via K=1 matmuls
                ztp = ps512.tile([128, TPB], f32, tag="sm", bufs=1, name=f"ztp{b}")
                for it in range(TPB):
                    nc.tensor.matmul(ztp[:, it:it + 1],
                                     zrow[:1, it * 128:(it + 1) * 128],
                                     ONE[:1, :1], start=(it == 0), stop=(it == TPB - 1))
                nc.scalar.copy(zcols[:, b * TPB:(b + 1) * TPB], ztp[:])
                # transpose Ht back to token-major, build h = Ht^T/Z + XV
                rz = work.tile([128, TPB], f32, tag="rz", bufs=2, name=f"rz{b}")
                nc.vector.reciprocal(rz[:], zcols[:, b * TPB:(b + 1) * TPB])
                for hq in range(2):
                    hp = ps512.tile([128, 512], f32, tag="q5", bufs=1, name=f"hq{b}_{hq}")
                    for k in range(4):
                        it = hq * 4 + k
                        nc.tensor.transpose(hp[:, k * 128:(k + 1) * 128],
                                            ht_sb[:, it * 128:(it + 1) * 128], EYE)
                    for k in range(4):
                        it = hq * 4 + k
                        t = b * TPB + it
                        nc.vector._custom_dve(
                            AFFINE_THEN_ADD,
                            out=hblob[:, t * 128:(t + 1) * 128],
                            in0=hp[:, k * 128:(k + 1) * 128],
                            in1=xv[:, t * 128:(t + 1) * 128],
                            s0=rz[:, it:it + 1], s1=0.0)

                # ---- LayerNorm for this batch's 8 tiles ----
                bs, be = b * TPB, (b + 1) * TPB
                for t in range(bs, be):
                    hs = hblob[:, t * 128:(t + 1) * 128]
                    nc.vector.reduce_sum(MU[:, t:t + 1], hs, axis=mybir.AxisListType.X)
                    sc2 = work.tile([128, 128], f32, tag="sqs2", bufs=2, name=f"sq2_{t}")
                    nc.vector.tensor_tensor(out=sc2[:], in0=hs, in1=hs, op=ALU.mult)
                    nc.vector.reduce_sum(SQ[:, t:t + 1], sc2[:], axis=mybir.AxisListType.X)
                MEAN = MU[:, bs:be]
                SQb = SQ[:, bs:be]
                msq = work.tile([128, TPB], f32, tag="msq", bufs=2, name=f"msq{b}")
                VAR = work.tile([128, TPB], f32, tag="var", bufs=2, name=f"var{b}")
                nc.vector.tensor_scalar_mul(MEAN, MU[:, bs:be], 1.0 / D)
                nc.vector.tensor_tensor(out=msq[:], in0=MEAN, in1=MEAN, op=ALU.mult)
                nc.vector.tensor_scalar(VAR[:], SQb, 1.0 / D, None, ALU.mult)
                nc.vector.tensor_tensor(out=VAR[:], in0=VAR[:], in1=msq[:], op=ALU.subtract)
                nc.vector.tensor_scalar_add(VAR[:], VAR[:], EPS_LN)
                STD = work.tile([128, TPB], f32, tag="std", bufs=2, name=f"std{b}")
                nc.scalar.activation(STD[:], VAR[:], AF.Sqrt)
                RSTD = work.tile([128, TPB], f32, tag="rstd", bufs=2, name=f"rstd{b}")
                nc.vector.reciprocal(RSTD[:], STD[:])
                for t in range(bs, be):
                    nc.vector.tensor_scalar(
                        OUT[:, t * 128:(t + 1) * 128], hblob[:, t * 128:(t + 1) * 128],
                        MEAN[:, t - bs:t - bs + 1], RSTD[:, t - bs:t - bs + 1],
                        ALU.subtract, ALU.mult)
                for t in range(bs, be):
                    sl = OUT[:, t * 128:(t + 1) * 128]
                    nc.gpsimd.tensor_tensor(out=sl, in0=sl, in1=GAM, op=ALU.mult)
                    nc.gpsimd.tensor_tensor(out=sl, in0=sl, in1=BET, op=ALU.add)
                nc.sync.dma_start(
                    yout[b * SEQ:(b + 1) * SEQ, :].rearrange("(t p) c -> p t c", p=128),
                    OUT[:, bs * 128:be * 128].rearrange("p (t c) -> p t c", c=128),
                )


            if debug:
                for nm, tl in [("d_xhat", xhat_t), ("d_y", y_t), ("d_v", v_r),
                               ("d_h", hblob), ("d_et", et[0]), ("d_ht", ht_sb),
                               ("d_zr", zrow)]:
                    dd = nc.dram_tensor(nm, list(tl.shape), f32,
                                        kind="ExternalOutput").ap()
                    if tl[:].dtype == f32r:
                        tmp = big.tile(list(tl.shape), f32, name=f"cv_{nm}")
                        nc.vector.tensor_copy(tmp[:], tl[:].bitcast(f32))
                        nc.gpsimd.dma_start(dd[:], tmp[:])
                    else:
                        nc.gpsimd.dma_start(dd[:], tl[:])
                dz = nc.dram_tensor("d_z", [128, TILES], f32, kind="ExternalOutput").ap()
                nc.gpsimd.dma_start(dz[:], zcols[:])
    nc.compile()
    return nc


def _get_nc():
    if "nc" not in _CACHE:
        _CACHE["nc"] = _build_program()
    return _CACHE["nc"]


def kernel(x, phi, Vw, Vb, gamma, beta):
    from concourse.bass_utils import run_bass_kernel_spmd

    x = np.asarray(x, dtype=np.float32)
    U = _build_U(np.asarray(phi)).astype(np.float32)

    cst = np.zeros((128, C_W), dtype=np.float32)
    cst[:, C_UT:C_UT + 128] = U.T
    cst[:, C_EYE:C_EYE + 128] = np.eye(128, dtype=np.float32)
    cst[:, C_VWT:C_VWT + 128] = np.asarray(Vw, np.float32).T
    cst[:, C_GAM:C_GAM + 128] = np.broadcast_to(np.asarray(gamma, np.float32), (128, 128))
    cst[:, C_BET:C_BET + 128] = np.broadcast_to(np.asarray(beta, np.float32), (128, 128))
    cst[:, C_VBR:C_VBR + 128] = np.broadcast_to(np.asarray(Vb, np.float32), (128, 128))
    cst[:, C_ONE] = 1.0

    nc = _get_nc()
    in_maps = []
    for c in range(N_CORES):
        xs = x[c * B_PER_CORE:(c + 1) * B_PER_CORE].reshape(B_PER_CORE * SEQ, D)
        in_maps.append({"xin": np.ascontiguousarray(xs), "cst": cst})
    res = run_bass_kernel_spmd(nc, in_maps, core_ids=list(range(N_CORES)))
    out = np.empty((BATCH, SEQ, D), dtype=np.float32)
    for c in range(N_CORES):
        out[c * B_PER_CORE:(c + 1) * B_PER_CORE] = (
            res.results[c]["yout"].reshape(B_PER_CORE, SEQ, D))
    return out
